# revision 12
# baseline (speedup 1.0000x reference)
"""MoE fusion kernel for Trainium2 (8 NeuronCores, two-phase sparse routing).

Structure
---------
Phase 1 (gate kernel, data-parallel over tokens): each core computes gate
logits for its 1024 tokens with a SINGLE bf16 pass of the big matmul
(max logit error ~7e-3), fully pipelined per-fo-tile weight DMA and
per-512-token-chunk moving-operand DMA, bf16 second matmul.  Tokens whose
top2/top3 logit gap is below THETA=2.5e-2 (~800 of 8192) are ambiguous at
that precision; the host recomputes their logits exactly (fp32, ~1 GMAC --
same spirit as the host softmax/top-k).  All other tokens route correctly
because their gap exceeds the worst-case device error by >1.7x; their
combine weights use the device logits (weight error <4e-3 -> output error
<2e-3).  ~42 us/pass on HW.

Host: softmax/top-2/renormalize, then slot assignment by exact bin-cover
DP.  Cost model: each token costs 432 PE cycles (=180 ns) and each
(weight-pass x token-chunk) costs ~48 ns of LDWEIGHTS serialized into the
self-loading matmul (measured; the PE reloads the stationary operand every
matmul), i.e. ~20.7 us per slot-chunk.  choose_profile() minimizes
J = 180*sum(SL) + 20700*sum(ceil(SL_j/512)) over slot-capacity profiles SL
subject to the 12 experts packing into 8 copies of SL with at most one
expert per slot (exact DP).  For the target load distribution this gives
SL=[512, 488, 480, 424, 168] (5 slots/core, 1.2% padding, 5 chunks).
Slots load each expert's weights once (14.2 MB/slot); 5 slots/core keeps
the weight stream (~200 us/core) well under the PE time (~475 us/core).

Phase 2 (expert kernel): each core runs its slots; per slot one expert's
weights stream in 1.6 MB fo-groups (6 for W1, 3 for W2, sync queue; bulk
activations on the gpsimd queue) and the MLP runs in bf16, feature-major,
weights stationary:

    x.T [1536, S] -> h.T = gelu(W1.T x.T) [3072, S] -> o.T = W2.T h.T [768, S]

Token chunks are even splits of S into ceil(S/512) pieces (PSUM bank
limit).  The final sigmoid folds into the combine weights as
out = w*sigmoid(o) = wb2 + wb2*tanh(0.5*o + 0.5*b2), wb2 = w/2, so only
the one 'gelu_and_others' ACT table set (gelu + tanh + copy) is ever
loaded.  The host scatter-adds the pre-weighted slot outputs into the
[N, E] result.  ~495 us/pass on HW.

Measurement note: per-pass times come from rep-NEFFs (body repeated R
times); R must be large enough that one dispatch carries >>3 ms of device
work or the burst measurement reads client dispatch cost instead of
device time (gate uses R=128).

A dense all-experts fallback kernel handles pathologically skewed routing.
"""

import math

import numpy as np

try:
    import concourse  # noqa: F401
except ImportError:  # pragma: no cover
    import sys

    sys.path.insert(0, "/opt/trn_rl_repo")

import concourse.bass as bass  # noqa: F401
import concourse.mybir as mybir
import concourse.tile as tile
from concourse import bacc
from concourse.bass_utils import run_bass_kernel_spmd

# Problem shapes (hardcoded per contest rules).
N, D, E, H, NE = 8192, 1536, 768, 3072, 12
NCORES = 8
T = N // NCORES  # 1024 tokens per core
P = 128
KO1 = D // P  # 12   k-tiles of the first expert matmul
FO1 = H // P  # 24   feature-tiles of h
KO2 = H // P  # 24   k-tiles of the second expert matmul
FO2 = E // P  # 6    feature-tiles of the output
GFO = E // P  # 6    feature-tiles of the gate hidden
TT = T // 512  # 2   512-token chunks of the gate moving operand

F32 = mybir.dt.float32
BF16 = mybir.dt.bfloat16
AF = mybir.ActivationFunctionType
OP = mybir.AluOpType

GELU = AF.Gelu  # test.py sim-mode substitutes Tanh (CoreSim lacks Gelu)

WG1 = 4  # fo-tiles per W1 DMA group (FO1/WG1 = 6 groups)
WG2 = 2  # fo2-tiles per W2 DMA group (FO2/WG2 = 3 groups)
NG1 = FO1 // WG1
NG2 = FO2 // WG2
THETA = 2.5e-2  # top2/3 gap below which the host recomputes exact logits
S_CAP = 1024  # beyond this the phase-2 working set won't fit SBUF -> dense


def _chunks(total, step=512):
    return [(a, min(a + step, total)) for a in range(0, total, step)]


def _even_chunks(S):
    """Split S into ceil(S/512) near-equal chunks (multiples of 8)."""
    k = -(-S // 512)
    out, a = [], 0
    for i in range(k):
        w = ((S - a) // (k - i) + 7) // 8 * 8 if i < k - 1 else S - a
        w = min(w, 512, S - a)
        out.append((a, a + w))
        a += w
    return out


def _erf(x):
    try:
        from scipy.special import erf as _serf  # noqa: PLC0415

        return _serf(x)
    except ImportError:
        u = np.frompyfunc(math.erf, 1, 1)
        return u(x.astype(np.float64)).astype(np.float32)


def _gelu32(x):
    x = x.astype(np.float32)
    return (0.5 * x * (1.0 + _erf(x / np.sqrt(np.float32(2.0))))).astype(np.float32)


# ======================================================================
# Dense all-experts fallback (unchanged from the robust baseline).
# ======================================================================


def _emit_dense(tc, aps):
    nc = tc.nc
    (xT, xTb, gw1, gb1, gw2, gb2r, w1e, b1e, w2e, b2e, iden, out) = aps

    import contextlib

    with contextlib.ExitStack() as ctx:
        pers = ctx.enter_context(tc.tile_pool(name="pers", bufs=1))
        xTb_s = pers.tile([P, KO1, T], BF16)
        nc.sync.dma_start(xTb_s[:], xTb)
        b1e_s = pers.tile([P, NE, FO1], F32)
        nc.sync.dma_start(b1e_s[:], b1e)
        b2e_s = pers.tile([P, NE, FO2], F32)
        nc.sync.dma_start(b2e_s[:], b2e)
        acc = pers.tile([P, FO2, T], F32)
        wT = pers.tile([NE, T], F32)
        ones_sb = pers.tile([1, P], F32)
        nc.vector.memset(ones_sb[:], 1.0)

        with (
            tc.tile_pool(name="gate_sb", bufs=1) as gsb,
            tc.tile_pool(name="gate_tmp", bufs=2) as gtmp,
            tc.tile_pool(name="gate_ps", bufs=2, space="PSUM") as gps,
            tc.tile_pool(name="gate_ps_small", bufs=2, space="PSUM") as gpss,
        ):
            xT_s = gsb.tile([P, KO1, T], F32)
            nc.sync.dma_start(xT_s[:], xT)
            gw1_s = gsb.tile([P, KO1, E], F32)
            nc.sync.dma_start(gw1_s[:], gw1)
            gb1_s = gsb.tile([P, GFO], F32)
            nc.sync.dma_start(gb1_s[:], gb1)
            gw2_s = gsb.tile([P, GFO, NE], F32)
            nc.sync.dma_start(gw2_s[:], gw2)
            gb2r_s = gsb.tile([P, NE], F32)
            nc.sync.dma_start(gb2r_s[:], gb2r)
            iden_s = gsb.tile([P, P], F32)
            nc.sync.dma_start(iden_s[:], iden)
            ghT = gsb.tile([P, GFO, T], F32)

            for fo in range(GFO):
                pg = gps.tile([P, T], F32, tag="gps")
                for t2 in range(TT):
                    for ko in range(KO1):
                        nc.tensor.matmul(
                            pg[:, t2 * 512 : (t2 + 1) * 512],
                            lhsT=gw1_s[:, ko, fo * P : (fo + 1) * P],
                            rhs=xT_s[:, ko, t2 * 512 : (t2 + 1) * 512],
                            start=(ko == 0),
                            stop=(ko == KO1 - 1),
                        )
                nc.scalar.activation(
                    ghT[:, fo, :], pg[:], GELU, bias=gb1_s[:, fo : fo + 1]
                )

            for tt in range(T // P):
                pl = gpss.tile([P, NE], F32, tag="gpl")
                for fo in range(GFO):
                    nc.tensor.matmul(
                        pl[:],
                        lhsT=ghT[:, fo, tt * P : (tt + 1) * P],
                        rhs=gw2_s[:, fo, :],
                        start=(fo == 0),
                        stop=(fo == GFO - 1),
                    )
                lt = gtmp.tile([P, NE], F32, tag="lt")
                nc.vector.tensor_tensor(lt[:], pl[:], gb2r_s[:], OP.add)
                m8 = gtmp.tile([P, 8], F32, tag="m8")
                nc.vector.max(m8[:], lt[:])
                dlt = gtmp.tile([P, 1], F32, tag="dlt")
                nc.vector.tensor_tensor(dlt[:], m8[:, 0:1], m8[:, 1:2], OP.subtract)
                w1v = gtmp.tile([P, 1], F32, tag="w1v")
                nc.scalar.activation(w1v[:], dlt[:], AF.Tanh, scale=0.5)
                nc.vector.tensor_scalar(w1v[:], w1v[:], 0.5, 0.5, OP.mult, OP.add)
                w2v = gtmp.tile([P, 1], F32, tag="w2v")
                nc.vector.tensor_scalar(w2v[:], w1v[:], -1.0, 1.0, OP.mult, OP.add)
                eq1 = gtmp.tile([P, NE], F32, tag="eq1")
                nc.vector.tensor_scalar(eq1[:], lt[:], m8[:, 0:1], None, OP.is_equal)
                nc.vector.tensor_scalar(eq1[:], eq1[:], w1v[:], None, OP.mult)
                eq2 = gtmp.tile([P, NE], F32, tag="eq2")
                nc.vector.tensor_scalar(eq2[:], lt[:], m8[:, 1:2], None, OP.is_equal)
                nc.vector.tensor_scalar(eq2[:], eq2[:], w2v[:], None, OP.mult)
                nc.vector.tensor_tensor(eq1[:], eq1[:], eq2[:], OP.add)
                ptw = gpss.tile([NE, P], F32, tag="gpt")
                nc.tensor.transpose(ptw[:], eq1[:], iden_s[:])
                nc.vector.tensor_copy(wT[:, tt * P : (tt + 1) * P], ptw[:])

        w1pool = ctx.enter_context(tc.tile_pool(name="w1p", bufs=2))
        w2pool = ctx.enter_context(tc.tile_pool(name="w2p", bufs=3))
        hpool = ctx.enter_context(tc.tile_pool(name="hp", bufs=FO1 + 4))
        wbpool = ctx.enter_context(tc.tile_pool(name="wbp", bufs=2))
        spool = ctx.enter_context(tc.tile_pool(name="sp", bufs=2))
        tpool = ctx.enter_context(tc.tile_pool(name="tp", bufs=2))
        psA = ctx.enter_context(tc.tile_pool(name="psA", bufs=2, space="PSUM"))
        psB = ctx.enter_context(tc.tile_pool(name="psB", bufs=2, space="PSUM"))

        for e in range(NE):
            wb = wbpool.tile([P, T], F32, tag="wb")
            wrow = wbpool.tile([1, T], F32, tag="wrow")
            nc.sync.dma_start(wrow[:], wT[e : e + 1, :])
            pwb = psA.tile([P, T], F32, tag="psA")
            for t2 in range(TT):
                nc.tensor.matmul(
                    pwb[:, t2 * 512 : (t2 + 1) * 512],
                    lhsT=ones_sb[:],
                    rhs=wrow[:, t2 * 512 : (t2 + 1) * 512],
                    start=True,
                    stop=True,
                )
            nc.vector.tensor_copy(wb[:], pwb[:])

            hts = []
            for fop in range(FO1 // 2):
                w1t = w1pool.tile([P, 2, KO1, P], BF16, tag="w1t")
                nc.sync.dma_start(w1t[:], w1e[e, fop])
                for q in range(2):
                    fo = 2 * fop + q
                    pa = psA.tile([P, T], F32, tag="psA")
                    for ko in range(KO1):
                        for t2 in range(TT):
                            nc.tensor.matmul(
                                pa[:, t2 * 512 : (t2 + 1) * 512],
                                lhsT=w1t[:, q, ko, :],
                                rhs=xTb_s[:, ko, t2 * 512 : (t2 + 1) * 512],
                                start=(ko == 0),
                                stop=(ko == KO1 - 1),
                            )
                    ht = hpool.tile([P, T], BF16, tag="ht")
                    nc.scalar.activation(
                        ht[:], pa[:], GELU, bias=b1e_s[:, e, fo : fo + 1]
                    )
                    hts.append(ht)

            for fop2 in range(FO2 // 2):
                w2t = w2pool.tile([P, 2, KO2, P], BF16, tag="w2t")
                nc.sync.dma_start(w2t[:], w2e[e, fop2])
                for q2 in range(2):
                    fo2 = 2 * fop2 + q2
                    pb = psB.tile([P, T], F32, tag="psB")
                    for ko in range(KO2):
                        for t2 in range(TT):
                            nc.tensor.matmul(
                                pb[:, t2 * 512 : (t2 + 1) * 512],
                                lhsT=w2t[:, q2, ko, :],
                                rhs=hts[ko][:, t2 * 512 : (t2 + 1) * 512],
                                start=(ko == 0),
                                stop=(ko == KO2 - 1),
                            )
                    st = spool.tile([P, T], F32, tag="st")
                    nc.scalar.activation(
                        st[:], pb[:], AF.Tanh, bias=b2e_s[:, e, fo2 : fo2 + 1], scale=0.5
                    )
                    if e == 0:
                        nc.vector.tensor_tensor(acc[:, fo2, :], st[:], wb[:], OP.mult)
                    else:
                        tmp = tpool.tile([P, T], F32, tag="tmp")
                        nc.vector.tensor_tensor(tmp[:], st[:], wb[:], OP.mult)
                        nc.vector.tensor_tensor(
                            acc[:, fo2, :], acc[:, fo2, :], tmp[:], OP.add
                        )

        for fo2 in range(FO2):
            fin = tpool.tile([P, T], F32, tag="fin")
            nc.vector.tensor_scalar(fin[:], acc[:, fo2, :], 0.5, 0.5, OP.mult, OP.add)
            nc.sync.dma_start(out[:, fo2, :], fin[:])


def build_nc():
    nc = bacc.Bacc(
        "TRN2", target_bir_lowering=False, debug=False, num_devices=NCORES
    )
    aps = (
        nc.dram_tensor("xT", [P, KO1, T], F32, kind="ExternalInput").ap(),
        nc.dram_tensor("xTb", [P, KO1, T], BF16, kind="ExternalInput").ap(),
        nc.dram_tensor("gw1", [P, KO1, E], F32, kind="ExternalInput").ap(),
        nc.dram_tensor("gb1", [P, GFO], F32, kind="ExternalInput").ap(),
        nc.dram_tensor("gw2", [P, GFO, NE], F32, kind="ExternalInput").ap(),
        nc.dram_tensor("gb2r", [P, NE], F32, kind="ExternalInput").ap(),
        nc.dram_tensor(
            "w1e", [NE, FO1 // 2, P, 2, KO1, P], BF16, kind="ExternalInput"
        ).ap(),
        nc.dram_tensor("b1e", [P, NE, FO1], F32, kind="ExternalInput").ap(),
        nc.dram_tensor(
            "w2e", [NE, FO2 // 2, P, 2, KO2, P], BF16, kind="ExternalInput"
        ).ap(),
        nc.dram_tensor("b2e", [P, NE, FO2], F32, kind="ExternalInput").ap(),
        nc.dram_tensor("iden", [P, P], F32, kind="ExternalInput").ap(),
        nc.dram_tensor("accT", [P, FO2, T], F32, kind="ExternalOutput").ap(),
    )
    with tile.TileContext(nc) as tc:
        _emit_dense(tc, aps)
    nc.compile()
    return nc


def prep_inputs(inputs):
    """Host-side sharding / relayout shared by dense + sparse paths."""
    bf16 = mybir.dt.np(BF16)
    combined = np.asarray(inputs["combined"], np.float32)
    gate_w1 = np.asarray(inputs["gate_w1"], np.float32)
    gate_b1 = np.asarray(inputs["gate_b1"], np.float32)
    gate_w2 = np.asarray(inputs["gate_w2"], np.float32)
    gate_b2 = np.asarray(inputs["gate_b2"], np.float32)
    ew1 = np.asarray(inputs["ew1"], np.float32)
    eb1 = np.asarray(inputs["eb1"], np.float32)
    ew2 = np.asarray(inputs["ew2"], np.float32)
    eb2 = np.asarray(inputs["eb2"], np.float32)

    gw1r = np.ascontiguousarray(gate_w1.reshape(KO1, P, E).transpose(1, 0, 2))
    # fo-major bf16 gate W1 for the pipelined per-fo-tile DMA
    gwhf = np.ascontiguousarray(
        gate_w1.reshape(KO1, P, GFO, P).transpose(2, 1, 0, 3)
    ).astype(bf16)
    shared = {
        "gw1": gw1r,
        "gwhf": gwhf,
        "gb1": np.ascontiguousarray(gate_b1.reshape(GFO, P).T),
        "gw2": np.ascontiguousarray(gate_w2.reshape(GFO, P, NE).transpose(1, 0, 2)),
        "gw2b": np.ascontiguousarray(
            gate_w2.reshape(GFO, P, NE).transpose(1, 0, 2)
        ).astype(bf16),
        "gb2r": np.ascontiguousarray(np.broadcast_to(gate_b2, (P, NE))),
        "gb2c": np.ascontiguousarray(gate_b2.reshape(NE, 1)),
        # dense-path fo-pair layouts
        "w1e": np.ascontiguousarray(
            ew1.reshape(NE, KO1, P, FO1 // 2, 2, P).transpose(0, 3, 2, 4, 1, 5)
        ).astype(bf16),
        "b1e": np.ascontiguousarray(eb1.reshape(NE, FO1, P).transpose(2, 0, 1)),
        "w2e": np.ascontiguousarray(
            ew2.reshape(NE, KO2, P, FO2 // 2, 2, P).transpose(0, 3, 2, 4, 1, 5)
        ).astype(bf16),
        "b2e": np.ascontiguousarray(
            (0.5 * eb2).reshape(NE, FO2, P).transpose(2, 0, 1)
        ),
        # sparse-path fo-group layouts
        "w1g": np.ascontiguousarray(
            ew1.reshape(NE, KO1, P, NG1, WG1, P).transpose(0, 3, 2, 4, 1, 5)
        ).astype(bf16),
        "w2g": np.ascontiguousarray(
            ew2.reshape(NE, KO2, P, NG2, WG2, P).transpose(0, 3, 2, 4, 1, 5)
        ).astype(bf16),
        "iden": np.eye(P, dtype=np.float32),
    }
    xTs, xTbs, xh2s = [], [], []
    for c in range(NCORES):
        xt = np.ascontiguousarray(
            combined[c * T : (c + 1) * T].T.reshape(KO1, P, T).transpose(1, 0, 2)
        )
        xTs.append(xt)
        xh = np.ascontiguousarray(xt.astype(bf16))
        xTbs.append(xh)
        # chunk-major bf16 tokens for the gate: [P, TT, KO1, 512]
        xh2s.append(
            np.ascontiguousarray(
                xh.reshape(P, KO1, TT, 512).transpose(0, 2, 1, 3)
            )
        )
    return shared, xTs, xTbs, xh2s


def gate_inmaps(shared, xh2s):
    return [
        {
            "xh": xh2s[c],
            "gwh": shared["gwhf"],
            "gb1": shared["gb1"],
            "gw2b": shared["gw2b"],
            "gb2r": shared["gb2c"],
        }
        for c in range(NCORES)
    ]


_NC_CACHE = {}


def kernel_dense(**inputs):
    if "nc" not in _NC_CACHE:
        _NC_CACHE["nc"] = build_nc()
    nc = _NC_CACHE["nc"]

    shared, xTs, xTbs, _ = prep_inputs(inputs)
    in_maps = [
        {**shared, "xT": xTs[c], "xTb": xTbs[c]} for c in range(NCORES)
    ]
    res = run_bass_kernel_spmd(nc, in_maps, core_ids=list(range(NCORES)))
    outs = res.results

    fused = np.empty((N, E), np.float32)
    for c in range(NCORES):
        accT = outs[c]["accT"]  # [P, FO2, T]
        fused[c * T : (c + 1) * T] = accT.transpose(2, 1, 0).reshape(T, E)
    return fused


# ======================================================================
# Sparse (true MoE routing) two-phase path.
# ======================================================================


def build_nc_gate(reps=1):
    """Gate kernel: single bf16 pass of mm1 + bf16 mm2, fully pipelined
    per-fo-tile weight DMA and per-512-token-chunk activation DMA."""
    nc = bacc.Bacc("TRN2", target_bir_lowering=False, debug=False, num_devices=NCORES)
    xh = nc.dram_tensor("xh", [P, TT, KO1, 512], BF16, kind="ExternalInput").ap()
    gwh = nc.dram_tensor("gwh", [GFO, P, KO1, P], BF16, kind="ExternalInput").ap()
    gb1 = nc.dram_tensor("gb1", [P, GFO], F32, kind="ExternalInput").ap()
    gw2b = nc.dram_tensor("gw2b", [P, GFO, NE], BF16, kind="ExternalInput").ap()
    gb2r = nc.dram_tensor("gb2r", [NE, 1], F32, kind="ExternalInput").ap()
    lg = nc.dram_tensor("lg", [NE, T], F32, kind="ExternalOutput").ap()

    with tile.TileContext(nc) as tc:
        with (
            tc.tile_pool(name="sb", bufs=2) as sb,
            tc.tile_pool(name="gh", bufs=2) as ghp,
            tc.tile_pool(name="tmp", bufs=2) as tmp,
            tc.tile_pool(name="ps", bufs=2, space="PSUM") as ps,
            tc.tile_pool(name="psl", bufs=2, space="PSUM") as psl,
        ):
            for _rep in range(reps):
                gwh_s = sb.tile([P, GFO, KO1, P], BF16, tag="gwh")
                xh_s = sb.tile([P, TT, KO1, 512], BF16, tag="xh")
                gb1_s = sb.tile([P, GFO], F32, tag="gb1")
                gw2b_s = sb.tile([P, GFO, NE], BF16, tag="gw2b")
                gb2r_s = sb.tile([NE, 1], F32, tag="gb2r")
                # pipelined head: first fo-tile + first token chunk land first
                nc.sync.dma_start(gwh_s[:, 0], gwh[0])
                nc.scalar.dma_start(xh_s[:, 0], xh[:, 0])
                for fo in range(1, GFO):
                    nc.sync.dma_start(gwh_s[:, fo], gwh[fo])
                nc.sync.dma_start(gw2b_s[:], gw2b)
                nc.sync.dma_start(gb1_s[:], gb1)
                nc.sync.dma_start(gb2r_s[:], gb2r)
                nc.scalar.dma_start(xh_s[:, 1], xh[:, 1])

                for t2 in range(TT):
                    ghT = ghp.tile([P, GFO, 512], BF16, tag="ghT")
                    for fo in range(GFO):
                        pg = ps.tile([P, 512], F32, tag="pg")
                        for ko in range(KO1):
                            nc.tensor.matmul(
                                pg[:],
                                lhsT=gwh_s[:, fo, ko, :],
                                rhs=xh_s[:, t2, ko, :],
                                start=(ko == 0),
                                stop=(ko == KO1 - 1),
                            )
                        nc.scalar.activation(
                            ghT[:, fo, :], pg[:], GELU, bias=gb1_s[:, fo : fo + 1]
                        )
                    pl = psl.tile([NE, 512], F32, tag="pl")
                    for kc in range(GFO):
                        nc.tensor.matmul(
                            pl[:],
                            lhsT=gw2b_s[:, kc, :],
                            rhs=ghT[:, kc, :],
                            start=(kc == 0),
                            stop=(kc == GFO - 1),
                        )
                    lt = tmp.tile([NE, 512], F32, tag="lt")
                    nc.vector.tensor_scalar(lt[:], pl[:], gb2r_s[:], None, OP.add)
                    nc.scalar.dma_start(lg[:, t2 * 512 : (t2 + 1) * 512], lt[:])
    nc.compile()
    return nc


def build_nc_exp(SL, reps=1):
    """Expert-phase kernel.  SL is the per-core slot-capacity profile (list of
    ints, each a multiple of 8, <= S_CAP); every core runs the same program
    with slot j sized SL[j].  Weights stream per-slot in fo-groups."""
    SL = list(SL)
    EXP = len(SL)
    assert all(8 <= c <= S_CAP and c % 8 == 0 for c in SL)
    Tc = sum(SL)
    offs = [0]
    for c in SL:
        offs.append(offs[-1] + c)
    Smax = max(SL)
    nc = bacc.Bacc("TRN2", target_bir_lowering=False, debug=False, num_devices=NCORES)
    xTe = nc.dram_tensor("xTe", [P, KO1 * Tc], BF16, kind="ExternalInput").ap()
    wrow = nc.dram_tensor("wrow", [1, Tc], F32, kind="ExternalInput").ap()
    w1s = nc.dram_tensor(
        "w1s", [EXP, NG1, P, WG1, KO1, P], BF16, kind="ExternalInput"
    ).ap()
    b1s = nc.dram_tensor("b1s", [P, EXP, FO1], F32, kind="ExternalInput").ap()
    w2s = nc.dram_tensor(
        "w2s", [EXP, NG2, P, WG2, KO2, P], BF16, kind="ExternalInput"
    ).ap()
    b2s = nc.dram_tensor("b2s", [P, EXP, FO2], F32, kind="ExternalInput").ap()
    oT = nc.dram_tensor("oT", [P, FO2, Tc], F32, kind="ExternalOutput").ap()

    import contextlib

    with tile.TileContext(nc) as tc:
        with contextlib.ExitStack() as ctx:
            pers = ctx.enter_context(tc.tile_pool(name="pers", bufs=1))
            persb = ctx.enter_context(tc.tile_pool(name="persb", bufs=2))
            psA = ctx.enter_context(tc.tile_pool(name="psA", bufs=2, space="PSUM"))
            w1pool = ctx.enter_context(tc.tile_pool(name="w1p", bufs=2))
            w2pool = ctx.enter_context(tc.tile_pool(name="w2p", bufs=2))
            hpool = ctx.enter_context(tc.tile_pool(name="hp", bufs=1))
            spool = ctx.enter_context(tc.tile_pool(name="sp", bufs=2))
            opool = ctx.enter_context(tc.tile_pool(name="op", bufs=2))
            psB = ctx.enter_context(tc.tile_pool(name="psB", bufs=2, space="PSUM"))
            for _rep in range(reps):
                xTe_s = pers.tile([P, KO1 * Tc], BF16, tag="xTe")
                b1s_s = persb.tile([P, EXP, FO1], F32, tag="b1s")
                b2s_s = persb.tile([P, EXP, FO2], F32, tag="b2s")
                wb2 = pers.tile([P, Tc], F32, tag="wb2")

                # small head DMAs first so the PE isn't stalled behind the
                # bulk xTe transfer (same queue is FIFO)
                ones_sb = pers.tile([1, P], F32, tag="ones")
                nc.vector.memset(ones_sb[:], 1.0)
                wrow_s = persb.tile([1, Tc], F32, tag="wrow")
                nc.gpsimd.dma_start(wrow_s[:], wrow)
                nc.gpsimd.dma_start(b1s_s[:], b1s)
                nc.gpsimd.dma_start(b2s_s[:], b2s)
                # slot-major transfer order: slot j's tokens land long before
                # the PE reaches slot j
                for j in range(EXP):
                    nc.gpsimd.dma_start(
                        xTe_s[:, KO1 * offs[j] : KO1 * offs[j + 1]],
                        xTe[:, KO1 * offs[j] : KO1 * offs[j + 1]],
                    )
                for a, b in _chunks(Tc):
                    pw = psA.tile([P, 512], F32, tag="psA")
                    nc.tensor.matmul(
                        pw[:, : b - a],
                        lhsT=ones_sb[:],
                        rhs=wrow_s[:, a:b],
                        start=True,
                        stop=True,
                    )
                    nc.scalar.mul(wb2[:, a:b], pw[:, : b - a], 0.5)  # wb2 = w/2

                for j in range(EXP):
                    S = SL[j]
                    t0 = offs[j]
                    cks = _even_chunks(S)
                    hbig = hpool.tile([P, KO2 * Smax], BF16, tag="ht")
                    for g in range(NG1):
                        w1t = w1pool.tile([P, WG1, KO1, P], BF16, tag="w1t")
                        nc.sync.dma_start(w1t[:], w1s[j, g])
                        for q in range(WG1):
                            fo = WG1 * g + q
                            for a, b in cks:
                                pa = psA.tile([P, 512], F32, tag="psA")
                                for ko in range(KO1):
                                    base = KO1 * t0 + ko * S
                                    nc.tensor.matmul(
                                        pa[:, : b - a],
                                        lhsT=w1t[:, q, ko, :],
                                        rhs=xTe_s[:, base + a : base + b],
                                        start=(ko == 0),
                                        stop=(ko == KO1 - 1),
                                    )
                                nc.scalar.activation(
                                    hbig[:, fo * S + a : fo * S + b],
                                    pa[:, : b - a],
                                    GELU,
                                    bias=b1s_s[:, j, fo : fo + 1],
                                )
                    for g2 in range(NG2):
                        w2t = w2pool.tile([P, WG2, KO2, P], BF16, tag="w2t")
                        nc.sync.dma_start(w2t[:], w2s[j, g2])
                        for q in range(WG2):
                            fo2 = WG2 * g2 + q
                            obuf = opool.tile([P, Smax], F32, tag="obuf")
                            for a, b in cks:
                                pb = psB.tile([P, 512], F32, tag="psB")
                                for ko in range(KO2):
                                    nc.tensor.matmul(
                                        pb[:, : b - a],
                                        lhsT=w2t[:, q, ko, :],
                                        rhs=hbig[:, ko * S + a : ko * S + b],
                                        start=(ko == 0),
                                        stop=(ko == KO2 - 1),
                                    )
                                st = spool.tile([P, 512], F32, tag="st")
                                # tanh(0.5*o + 0.5*b2)  (b2s input is pre-halved)
                                nc.scalar.activation(
                                    st[:, : b - a],
                                    pb[:, : b - a],
                                    AF.Tanh,
                                    bias=b2s_s[:, j, fo2 : fo2 + 1],
                                    scale=0.5,
                                )
                                # out = w*sigmoid(o) = wb2 + wb2*tanh
                                nc.vector.tensor_tensor(
                                    obuf[:, a:b],
                                    st[:, : b - a],
                                    wb2[:, t0 + a : t0 + b],
                                    OP.mult,
                                )
                                nc.vector.tensor_tensor(
                                    obuf[:, a:b],
                                    obuf[:, a:b],
                                    wb2[:, t0 + a : t0 + b],
                                    OP.add,
                                )
                            nc.gpsimd.dma_start(
                                oT[:, fo2, t0 : t0 + S], obuf[:, :S]
                            )
    nc.compile()
    return nc


def _host_exact_logits(inputs, rows):
    """Exact fp32 gate logits for the given token rows."""
    c = np.asarray(inputs["combined"], np.float32)[rows]
    gh = _gelu32(c @ np.asarray(inputs["gate_w1"], np.float32)
                 + np.asarray(inputs["gate_b1"], np.float32))
    return (gh @ np.asarray(inputs["gate_w2"], np.float32)
            + np.asarray(inputs["gate_b2"], np.float32))


# ---------- slot-profile selection: minimize PE cost under bin packing ----
#
# Cost model (per core, ns): each token costs 432 PE cycles = 180 ns, and
# each (weight-pass x token-chunk) costs a flat ~48 ns of serialized
# LDWEIGHTS in the self-loading matmul => 432*48 ~ 20.7 us per slot-chunk.
# So J(SL) = 180*sum(SL) + 20700*sum(ceil(SL_j/512)), minimized subject to
# the experts packing into 8 copies of SL with <=1 expert per bin.

_PROFILE_SEEDS = [(512, 488, 480, 424, 168)]  # known-good for the target dist


def _bundles(c, SL):
    """Pareto-minimal bin bundles (per-size counts) covering c tokens."""
    import itertools

    E = len(SL)
    rng = [range(0, min(NCORES, -(-c // SL[j])) + 1) for j in range(E)]
    combos = []
    for n in itertools.product(*rng):
        if sum(n) <= NCORES and sum(nj * s for nj, s in zip(n, SL)) >= c:
            combos.append(n)
    combos.sort(key=sum)
    keep = []
    for n in combos:
        if not any(all(m[j] <= n[j] for j in range(E)) and m != n for m in keep):
            keep.append(n)
    return keep


def _pack(cnt, SL):
    """Exact bin-cover DP.  Returns per-expert bundles (counts per size
    class) or None if the experts don't fit 8 copies of SL."""
    E = len(SL)
    if E > 6:
        return None
    order = np.argsort(-np.asarray(cnt))
    dims = (NCORES + 1,) * E
    nstates = int(np.prod(dims))
    reach = np.zeros(nstates, bool)
    reach[np.ravel_multi_index((NCORES,) * E, dims)] = True
    BL, layers = [], [reach]
    for e in order:
        B = _bundles(int(cnt[e]), SL)
        if not B:
            return None
        BL.append(B)
        idx = np.nonzero(reach)[0]
        if not len(idx):
            return None
        coords = np.stack(np.unravel_index(idx, dims), 1)
        new = np.zeros(nstates, bool)
        for b in B:
            c2 = coords - b
            ok = (c2 >= 0).all(1)
            if ok.any():
                new[np.ravel_multi_index(c2[ok].T.tolist(), dims)] = True
        reach = new
        layers.append(reach)
    if not reach.any():
        return None
    fin = np.nonzero(reach)[0][0]
    state = np.array(np.unravel_index(fin, dims))
    rev = []
    for i in range(len(order) - 1, -1, -1):
        found = None
        for b in BL[i]:
            p = state + b
            if (p <= NCORES).all() and layers[i][np.ravel_multi_index(p.tolist(), dims)]:
                found = b
                state = p
                break
        assert found is not None
        rev.append(found)
    rev.reverse()
    asg = [None] * NE
    for i, e in enumerate(order):
        asg[int(e)] = rev[i]
    return asg


def _waterfill_profile(cnt, EXP):
    """Near-equal waterfill pieces -> sorted groups of 8 -> profile + direct
    assignment (always feasible)."""
    k = np.ones(NE, np.int64)
    for _ in range(NCORES * EXP - NE):
        j = np.argmax(-(-cnt // k))
        k[j] += 1
    pieces = []
    for e in range(NE):
        base, rem = divmod(int(cnt[e]), int(k[e]))
        for i in range(int(k[e])):
            pieces.append((base + (1 if i < rem else 0), e))
    pieces.sort(key=lambda t: -t[0])
    SL = tuple(
        max(8, -(-max(pieces[j * NCORES + c][0] for c in range(NCORES)) // 8) * 8)
        for j in range(EXP)
    )
    asg = [[0] * EXP for _ in range(NE)]
    for j in range(EXP):
        for c in range(NCORES):
            asg[pieces[j * NCORES + c][1]][j] += 1
    return SL, asg


def _profile_cost(SL):
    return 180 * sum(SL) + 20700 * sum(-(-s // 512) for s in SL)


def choose_profile(cnt):
    """Pick the cheapest feasible slot profile for the given expert loads."""
    best = None
    for SL in _PROFILE_SEEDS:
        if max(SL) > S_CAP:
            continue
        asg = _pack(cnt, SL)
        if asg is not None:
            j = _profile_cost(SL)
            if best is None or j < best[0]:
                best = (j, tuple(SL), asg)
    for EXP in (5, 4, 6, 8):
        SL, asg = _waterfill_profile(cnt, EXP)
        if max(SL) > S_CAP:
            continue
        j = _profile_cost(SL)
        if best is None or j < best[0]:
            best = (j, SL, asg)
    if best is None:
        return None, None
    return list(best[1]), best[2]


def route(logits):
    """Host softmax/top-2/normalize + slot assignment via profile packing.

    Returns (slots, SL): slots[c][j] = (expert, token_idx[SL_j], weight[SL_j])
    padded with token 0 / weight 0."""
    lg = logits.astype(np.float32)
    m = lg.max(axis=1, keepdims=True)
    p = np.exp(lg - m)
    p /= p.sum(axis=1, keepdims=True)
    order = np.argsort(-p, axis=1, kind="stable")
    i1, i2 = order[:, 0], order[:, 1]
    r = np.arange(lg.shape[0])
    w1 = p[r, i1]
    w2 = p[r, i2]
    s = w1 + w2
    w1, w2 = w1 / s, w2 / s

    toks, wts = [], []
    for e in range(NE):
        t1 = np.nonzero(i1 == e)[0]
        t2 = np.nonzero(i2 == e)[0]
        toks.append(np.concatenate([t1, t2]))
        wts.append(np.concatenate([w1[t1], w2[t2]]).astype(np.float32))
    cnt = np.array([len(t) for t in toks])

    SL, asg = choose_profile(cnt)
    if SL is None:
        return None, None
    EXP = len(SL)
    # instantiate bins: class j has 8 instances (one per core), round-robin
    nxt = [0] * EXP  # next free core per class
    slots = [[None] * EXP for _ in range(NCORES)]
    for e in range(NE):
        pos = 0
        # fill this expert's bins largest-class-first so the partial bin is
        # the smallest one
        for j in range(EXP):
            for _ in range(asg[e][j]):
                c = nxt[j]
                nxt[j] += 1
                take = max(0, min(SL[j], cnt[e] - pos))
                tt = np.zeros(SL[j], np.int64)
                ww = np.zeros(SL[j], np.float32)
                tt[:take] = toks[e][pos : pos + take]
                ww[:take] = wts[e][pos : pos + take]
                slots[c][j] = (e, tt, ww)
                pos += take
        assert pos >= cnt[e], f"expert {e}: {pos} < {cnt[e]}"
    # unused bins -> padding (expert 0, zero weights)
    for j in range(EXP):
        for c in range(nxt[j], NCORES):
            slots[c][j] = (
                0,
                np.zeros(SL[j], np.int64),
                np.zeros(SL[j], np.float32),
            )
    return slots, SL


def build_emaps(inputs, shared, slots, SL):
    bf16 = mybir.dt.np(BF16)
    EXP = len(SL)
    Tc = sum(SL)
    combined = np.asarray(inputs["combined"], np.float32)
    emaps = []
    for c in range(NCORES):
        eids = [slots[c][j][0] for j in range(EXP)]
        ws = np.concatenate([slots[c][j][2] for j in range(EXP)])
        blocks = []
        for j in range(EXP):
            xg = combined[slots[c][j][1]]  # [SL[j], D]
            blocks.append(
                xg.T.reshape(KO1, P, SL[j]).transpose(1, 0, 2).reshape(P, KO1 * SL[j])
            )
        emaps.append(
            {
                "xTe": np.ascontiguousarray(np.concatenate(blocks, axis=1)).astype(
                    bf16
                ),
                "wrow": ws.reshape(1, Tc).astype(np.float32),
                "w1s": np.ascontiguousarray(shared["w1g"][eids]),
                "b1s": np.ascontiguousarray(shared["b1e"][:, eids, :]),
                "w2s": np.ascontiguousarray(shared["w2g"][eids]),
                "b2s": np.ascontiguousarray(shared["b2e"][:, eids, :]),
            }
        )
    return emaps


def kernel_sparse(**inputs):
    shared, xTs, _, xh2s = prep_inputs(inputs)

    if "gate" not in _NC_CACHE:
        _NC_CACHE["gate"] = build_nc_gate()
    ncg = _NC_CACHE["gate"]
    gmaps = gate_inmaps(shared, xh2s)
    gres = run_bass_kernel_spmd(ncg, gmaps, core_ids=list(range(NCORES)))
    logits = np.concatenate(
        [gres.results[c]["lg"].T for c in range(NCORES)]
    )

    # host fixup: recompute ambiguous tokens (top2/3 gap under THETA) exactly
    srt = np.sort(logits, axis=1)
    amb = np.nonzero((srt[:, -2] - srt[:, -3]) < THETA)[0]
    if len(amb):
        logits[amb] = _host_exact_logits(inputs, amb)

    slots, SL = route(logits)
    if SL is None or max(SL) > S_CAP or sum(SL) > 3000:  # pathological routing
        return kernel_dense(**inputs)
    EXP = len(SL)
    Tc = sum(SL)
    offs = [0]
    for ccap in SL:
        offs.append(offs[-1] + ccap)

    key = ("exp", tuple(SL))
    if key not in _NC_CACHE:
        _NC_CACHE[key] = build_nc_exp(SL)
    nce = _NC_CACHE[key]

    emaps = build_emaps(inputs, shared, slots, SL)
    _NC_CACHE["last_emaps"] = emaps
    _NC_CACHE["last_SL"] = SL
    eres = run_bass_kernel_spmd(nce, emaps, core_ids=list(range(NCORES)))

    fused = np.zeros((N, E), np.float32)
    for c in range(NCORES):
        rows = eres.results[c]["oT"].transpose(2, 1, 0).reshape(Tc, E)
        for j in range(EXP):
            # np.add.at: padding reuses token 0 with an all-zero weight row
            np.add.at(fused, slots[c][j][1], rows[offs[j] : offs[j + 1]])
    return fused


MODE = "sparse"


def kernel(**inputs):
    if MODE == "sparse":
        try:
            return kernel_sparse(**inputs)
        except Exception:
            return kernel_dense(**inputs)
    return kernel_dense(**inputs)


if __name__ == "__main__":  # dev smoke test only; harness imports kernel()
    import reference  # noqa: PLC0415 -- not needed when imported as a module

    inputs = {k: np.asarray(v) for k, v in reference.setup_inputs().items()}
    out = kernel(**inputs)
    print(out.shape, out.dtype)


# revision 13
# speedup vs baseline: 1.0206x; 1.0206x over previous
"""MoE fusion kernel for Trainium2 (8 NeuronCores, two-phase sparse routing).

Structure
---------
Phase 1 (gate kernel, data-parallel over tokens): each core computes gate
logits for its 1024 tokens with a SINGLE bf16 pass of the big matmul
(max logit error ~7e-3), fully pipelined per-fo-tile weight DMA and
per-512-token-chunk moving-operand DMA, bf16 second matmul.  Tokens whose
top2/top3 logit gap is below THETA=2.5e-2 (~800 of 8192) are ambiguous at
that precision; the host recomputes their logits exactly (fp32, ~1 GMAC --
same spirit as the host softmax/top-k).  All other tokens route correctly
because their gap exceeds the worst-case device error by >1.7x; their
combine weights use the device logits (weight error <4e-3 -> output error
<2e-3).  ~42 us/pass on HW.

Host: softmax/top-2/renormalize, then slot assignment by exact bin-cover
DP.  Cost model: each token costs 432 PE cycles (=180 ns) and each
(weight-pass x token-chunk) costs ~48 ns of LDWEIGHTS serialized into the
self-loading matmul (measured; the PE reloads the stationary operand every
matmul), i.e. ~20.7 us per slot-chunk.  choose_profile() minimizes
J = 180*sum(SL) + 20700*sum(ceil(SL_j/512)) over slot-capacity profiles SL
subject to the 12 experts packing into 8 copies of SL with at most one
expert per slot (exact DP).  For the target load distribution this gives
SL=[512, 488, 480, 424, 168] (5 slots/core, 1.2% padding, 5 chunks).
Slots load each expert's weights once (14.2 MB/slot); 5 slots/core keeps
the weight stream (~200 us/core) well under the PE time (~475 us/core).

Phase 2 (expert kernel): each core runs its slots; per slot one expert's
weights stream in 1.6 MB fo-groups (6 for W1, 3 for W2, sync queue; bulk
activations on the gpsimd queue) and the MLP runs in bf16, feature-major,
weights stationary:

    x.T [1536, S] -> h.T = gelu(W1.T x.T) [3072, S] -> o.T = W2.T h.T [768, S]

Token chunks are even splits of S into ceil(S/512) pieces (PSUM bank
limit).  The final sigmoid folds into the combine weights as
out = w*sigmoid(o) = wb2 + wb2*tanh(0.5*o + 0.5*b2), wb2 = w/2, so only
the one 'gelu_and_others' ACT table set (gelu + tanh + copy) is ever
loaded.  The host scatter-adds the pre-weighted slot outputs into the
[N, E] result.  ~495 us/pass on HW.

Measurement note: per-pass times come from rep-NEFFs (body repeated R
times); R must be large enough that one dispatch carries >>3 ms of device
work or the burst measurement reads client dispatch cost instead of
device time (gate uses R=128).

A dense all-experts fallback kernel handles pathologically skewed routing.
"""

import math

import numpy as np

try:
    import concourse  # noqa: F401
except ImportError:  # pragma: no cover
    import sys

    sys.path.insert(0, "/opt/trn_rl_repo")

import concourse.bass as bass  # noqa: F401
import concourse.mybir as mybir
import concourse.tile as tile
from concourse import bacc
from concourse.bass_utils import run_bass_kernel_spmd

# Problem shapes (hardcoded per contest rules).
N, D, E, H, NE = 8192, 1536, 768, 3072, 12
NCORES = 8
T = N // NCORES  # 1024 tokens per core
P = 128
KO1 = D // P  # 12   k-tiles of the first expert matmul
FO1 = H // P  # 24   feature-tiles of h
KO2 = H // P  # 24   k-tiles of the second expert matmul
FO2 = E // P  # 6    feature-tiles of the output
GFO = E // P  # 6    feature-tiles of the gate hidden
TT = T // 512  # 2   512-token chunks of the gate moving operand

F32 = mybir.dt.float32
BF16 = mybir.dt.bfloat16
AF = mybir.ActivationFunctionType
OP = mybir.AluOpType

GELU = AF.Gelu  # test.py sim-mode substitutes Tanh (CoreSim lacks Gelu)

WG1 = 4  # fo-tiles per W1 DMA group (FO1/WG1 = 6 groups)
WG2 = 2  # fo2-tiles per W2 DMA group (FO2/WG2 = 3 groups)
NG1 = FO1 // WG1
NG2 = FO2 // WG2
THETA = 2.5e-2  # top2/3 gap below which the host recomputes exact logits
S_CAP = 1024  # beyond this the phase-2 working set won't fit SBUF -> dense


def _chunks(total, step=512):
    return [(a, min(a + step, total)) for a in range(0, total, step)]


def _even_chunks(S):
    """Split S into ceil(S/512) near-equal chunks (multiples of 8)."""
    k = -(-S // 512)
    out, a = [], 0
    for i in range(k):
        w = ((S - a) // (k - i) + 7) // 8 * 8 if i < k - 1 else S - a
        w = min(w, 512, S - a)
        out.append((a, a + w))
        a += w
    return out


def _erf(x):
    try:
        from scipy.special import erf as _serf  # noqa: PLC0415

        return _serf(x)
    except ImportError:
        u = np.frompyfunc(math.erf, 1, 1)
        return u(x.astype(np.float64)).astype(np.float32)


def _gelu32(x):
    x = x.astype(np.float32)
    return (0.5 * x * (1.0 + _erf(x / np.sqrt(np.float32(2.0))))).astype(np.float32)


# ======================================================================
# Dense all-experts fallback (unchanged from the robust baseline).
# ======================================================================


def _emit_dense(tc, aps):
    nc = tc.nc
    (xT, xTb, gw1, gb1, gw2, gb2r, w1e, b1e, w2e, b2e, iden, out) = aps

    import contextlib

    with contextlib.ExitStack() as ctx:
        pers = ctx.enter_context(tc.tile_pool(name="pers", bufs=1))
        xTb_s = pers.tile([P, KO1, T], BF16)
        nc.sync.dma_start(xTb_s[:], xTb)
        b1e_s = pers.tile([P, NE, FO1], F32)
        nc.sync.dma_start(b1e_s[:], b1e)
        b2e_s = pers.tile([P, NE, FO2], F32)
        nc.sync.dma_start(b2e_s[:], b2e)
        acc = pers.tile([P, FO2, T], F32)
        wT = pers.tile([NE, T], F32)
        ones_sb = pers.tile([1, P], F32)
        nc.vector.memset(ones_sb[:], 1.0)

        with (
            tc.tile_pool(name="gate_sb", bufs=1) as gsb,
            tc.tile_pool(name="gate_tmp", bufs=2) as gtmp,
            tc.tile_pool(name="gate_ps", bufs=2, space="PSUM") as gps,
            tc.tile_pool(name="gate_ps_small", bufs=2, space="PSUM") as gpss,
        ):
            xT_s = gsb.tile([P, KO1, T], F32)
            nc.sync.dma_start(xT_s[:], xT)
            gw1_s = gsb.tile([P, KO1, E], F32)
            nc.sync.dma_start(gw1_s[:], gw1)
            gb1_s = gsb.tile([P, GFO], F32)
            nc.sync.dma_start(gb1_s[:], gb1)
            gw2_s = gsb.tile([P, GFO, NE], F32)
            nc.sync.dma_start(gw2_s[:], gw2)
            gb2r_s = gsb.tile([P, NE], F32)
            nc.sync.dma_start(gb2r_s[:], gb2r)
            iden_s = gsb.tile([P, P], F32)
            nc.sync.dma_start(iden_s[:], iden)
            ghT = gsb.tile([P, GFO, T], F32)

            for fo in range(GFO):
                pg = gps.tile([P, T], F32, tag="gps")
                for t2 in range(TT):
                    for ko in range(KO1):
                        nc.tensor.matmul(
                            pg[:, t2 * 512 : (t2 + 1) * 512],
                            lhsT=gw1_s[:, ko, fo * P : (fo + 1) * P],
                            rhs=xT_s[:, ko, t2 * 512 : (t2 + 1) * 512],
                            start=(ko == 0),
                            stop=(ko == KO1 - 1),
                        )
                nc.scalar.activation(
                    ghT[:, fo, :], pg[:], GELU, bias=gb1_s[:, fo : fo + 1]
                )

            for tt in range(T // P):
                pl = gpss.tile([P, NE], F32, tag="gpl")
                for fo in range(GFO):
                    nc.tensor.matmul(
                        pl[:],
                        lhsT=ghT[:, fo, tt * P : (tt + 1) * P],
                        rhs=gw2_s[:, fo, :],
                        start=(fo == 0),
                        stop=(fo == GFO - 1),
                    )
                lt = gtmp.tile([P, NE], F32, tag="lt")
                nc.vector.tensor_tensor(lt[:], pl[:], gb2r_s[:], OP.add)
                m8 = gtmp.tile([P, 8], F32, tag="m8")
                nc.vector.max(m8[:], lt[:])
                dlt = gtmp.tile([P, 1], F32, tag="dlt")
                nc.vector.tensor_tensor(dlt[:], m8[:, 0:1], m8[:, 1:2], OP.subtract)
                w1v = gtmp.tile([P, 1], F32, tag="w1v")
                nc.scalar.activation(w1v[:], dlt[:], AF.Tanh, scale=0.5)
                nc.vector.tensor_scalar(w1v[:], w1v[:], 0.5, 0.5, OP.mult, OP.add)
                w2v = gtmp.tile([P, 1], F32, tag="w2v")
                nc.vector.tensor_scalar(w2v[:], w1v[:], -1.0, 1.0, OP.mult, OP.add)
                eq1 = gtmp.tile([P, NE], F32, tag="eq1")
                nc.vector.tensor_scalar(eq1[:], lt[:], m8[:, 0:1], None, OP.is_equal)
                nc.vector.tensor_scalar(eq1[:], eq1[:], w1v[:], None, OP.mult)
                eq2 = gtmp.tile([P, NE], F32, tag="eq2")
                nc.vector.tensor_scalar(eq2[:], lt[:], m8[:, 1:2], None, OP.is_equal)
                nc.vector.tensor_scalar(eq2[:], eq2[:], w2v[:], None, OP.mult)
                nc.vector.tensor_tensor(eq1[:], eq1[:], eq2[:], OP.add)
                ptw = gpss.tile([NE, P], F32, tag="gpt")
                nc.tensor.transpose(ptw[:], eq1[:], iden_s[:])
                nc.vector.tensor_copy(wT[:, tt * P : (tt + 1) * P], ptw[:])

        w1pool = ctx.enter_context(tc.tile_pool(name="w1p", bufs=2))
        w2pool = ctx.enter_context(tc.tile_pool(name="w2p", bufs=3))
        hpool = ctx.enter_context(tc.tile_pool(name="hp", bufs=FO1 + 4))
        wbpool = ctx.enter_context(tc.tile_pool(name="wbp", bufs=2))
        spool = ctx.enter_context(tc.tile_pool(name="sp", bufs=2))
        tpool = ctx.enter_context(tc.tile_pool(name="tp", bufs=2))
        psA = ctx.enter_context(tc.tile_pool(name="psA", bufs=3, space="PSUM"))
        psB = ctx.enter_context(tc.tile_pool(name="psB", bufs=3, space="PSUM"))

        for e in range(NE):
            wb = wbpool.tile([P, T], F32, tag="wb")
            wrow = wbpool.tile([1, T], F32, tag="wrow")
            nc.sync.dma_start(wrow[:], wT[e : e + 1, :])
            pwb = psA.tile([P, T], F32, tag="psA")
            for t2 in range(TT):
                nc.tensor.matmul(
                    pwb[:, t2 * 512 : (t2 + 1) * 512],
                    lhsT=ones_sb[:],
                    rhs=wrow[:, t2 * 512 : (t2 + 1) * 512],
                    start=True,
                    stop=True,
                )
            nc.vector.tensor_copy(wb[:], pwb[:])

            hts = []
            for fop in range(FO1 // 2):
                w1t = w1pool.tile([P, 2, KO1, P], BF16, tag="w1t")
                nc.sync.dma_start(w1t[:], w1e[e, fop])
                for q in range(2):
                    fo = 2 * fop + q
                    pa = psA.tile([P, T], F32, tag="psA")
                    for ko in range(KO1):
                        for t2 in range(TT):
                            nc.tensor.matmul(
                                pa[:, t2 * 512 : (t2 + 1) * 512],
                                lhsT=w1t[:, q, ko, :],
                                rhs=xTb_s[:, ko, t2 * 512 : (t2 + 1) * 512],
                                start=(ko == 0),
                                stop=(ko == KO1 - 1),
                            )
                    ht = hpool.tile([P, T], BF16, tag="ht")
                    nc.scalar.activation(
                        ht[:], pa[:], GELU, bias=b1e_s[:, e, fo : fo + 1]
                    )
                    hts.append(ht)

            for fop2 in range(FO2 // 2):
                w2t = w2pool.tile([P, 2, KO2, P], BF16, tag="w2t")
                nc.sync.dma_start(w2t[:], w2e[e, fop2])
                for q2 in range(2):
                    fo2 = 2 * fop2 + q2
                    pb = psB.tile([P, T], F32, tag="psB")
                    for ko in range(KO2):
                        for t2 in range(TT):
                            nc.tensor.matmul(
                                pb[:, t2 * 512 : (t2 + 1) * 512],
                                lhsT=w2t[:, q2, ko, :],
                                rhs=hts[ko][:, t2 * 512 : (t2 + 1) * 512],
                                start=(ko == 0),
                                stop=(ko == KO2 - 1),
                            )
                    st = spool.tile([P, T], F32, tag="st")
                    nc.scalar.activation(
                        st[:], pb[:], AF.Tanh, bias=b2e_s[:, e, fo2 : fo2 + 1], scale=0.5
                    )
                    if e == 0:
                        nc.vector.tensor_tensor(acc[:, fo2, :], st[:], wb[:], OP.mult)
                    else:
                        tmp = tpool.tile([P, T], F32, tag="tmp")
                        nc.vector.tensor_tensor(tmp[:], st[:], wb[:], OP.mult)
                        nc.vector.tensor_tensor(
                            acc[:, fo2, :], acc[:, fo2, :], tmp[:], OP.add
                        )

        for fo2 in range(FO2):
            fin = tpool.tile([P, T], F32, tag="fin")
            nc.vector.tensor_scalar(fin[:], acc[:, fo2, :], 0.5, 0.5, OP.mult, OP.add)
            nc.sync.dma_start(out[:, fo2, :], fin[:])


def build_nc():
    nc = bacc.Bacc(
        "TRN2", target_bir_lowering=False, debug=False, num_devices=NCORES
    )
    aps = (
        nc.dram_tensor("xT", [P, KO1, T], F32, kind="ExternalInput").ap(),
        nc.dram_tensor("xTb", [P, KO1, T], BF16, kind="ExternalInput").ap(),
        nc.dram_tensor("gw1", [P, KO1, E], F32, kind="ExternalInput").ap(),
        nc.dram_tensor("gb1", [P, GFO], F32, kind="ExternalInput").ap(),
        nc.dram_tensor("gw2", [P, GFO, NE], F32, kind="ExternalInput").ap(),
        nc.dram_tensor("gb2r", [P, NE], F32, kind="ExternalInput").ap(),
        nc.dram_tensor(
            "w1e", [NE, FO1 // 2, P, 2, KO1, P], BF16, kind="ExternalInput"
        ).ap(),
        nc.dram_tensor("b1e", [P, NE, FO1], F32, kind="ExternalInput").ap(),
        nc.dram_tensor(
            "w2e", [NE, FO2 // 2, P, 2, KO2, P], BF16, kind="ExternalInput"
        ).ap(),
        nc.dram_tensor("b2e", [P, NE, FO2], F32, kind="ExternalInput").ap(),
        nc.dram_tensor("iden", [P, P], F32, kind="ExternalInput").ap(),
        nc.dram_tensor("accT", [P, FO2, T], F32, kind="ExternalOutput").ap(),
    )
    with tile.TileContext(nc) as tc:
        _emit_dense(tc, aps)
    nc.compile()
    return nc


def prep_inputs(inputs):
    """Host-side sharding / relayout shared by dense + sparse paths."""
    bf16 = mybir.dt.np(BF16)
    combined = np.asarray(inputs["combined"], np.float32)
    gate_w1 = np.asarray(inputs["gate_w1"], np.float32)
    gate_b1 = np.asarray(inputs["gate_b1"], np.float32)
    gate_w2 = np.asarray(inputs["gate_w2"], np.float32)
    gate_b2 = np.asarray(inputs["gate_b2"], np.float32)
    ew1 = np.asarray(inputs["ew1"], np.float32)
    eb1 = np.asarray(inputs["eb1"], np.float32)
    ew2 = np.asarray(inputs["ew2"], np.float32)
    eb2 = np.asarray(inputs["eb2"], np.float32)

    gw1r = np.ascontiguousarray(gate_w1.reshape(KO1, P, E).transpose(1, 0, 2))
    # fo-major bf16 gate W1 for the pipelined per-fo-tile DMA
    gwhf = np.ascontiguousarray(
        gate_w1.reshape(KO1, P, GFO, P).transpose(2, 1, 0, 3)
    ).astype(bf16)
    shared = {
        "gw1": gw1r,
        "gwhf": gwhf,
        "gb1": np.ascontiguousarray(gate_b1.reshape(GFO, P).T),
        "gw2": np.ascontiguousarray(gate_w2.reshape(GFO, P, NE).transpose(1, 0, 2)),
        "gw2b": np.ascontiguousarray(
            gate_w2.reshape(GFO, P, NE).transpose(1, 0, 2)
        ).astype(bf16),
        "gb2r": np.ascontiguousarray(np.broadcast_to(gate_b2, (P, NE))),
        "gb2c": np.ascontiguousarray(gate_b2.reshape(NE, 1)),
        # dense-path fo-pair layouts
        "w1e": np.ascontiguousarray(
            ew1.reshape(NE, KO1, P, FO1 // 2, 2, P).transpose(0, 3, 2, 4, 1, 5)
        ).astype(bf16),
        "b1e": np.ascontiguousarray(eb1.reshape(NE, FO1, P).transpose(2, 0, 1)),
        "w2e": np.ascontiguousarray(
            ew2.reshape(NE, KO2, P, FO2 // 2, 2, P).transpose(0, 3, 2, 4, 1, 5)
        ).astype(bf16),
        "b2e": np.ascontiguousarray(
            (0.5 * eb2).reshape(NE, FO2, P).transpose(2, 0, 1)
        ),
        # sparse-path fo-group layouts
        "w1g": np.ascontiguousarray(
            ew1.reshape(NE, KO1, P, NG1, WG1, P).transpose(0, 3, 2, 4, 1, 5)
        ).astype(bf16),
        "w2g": np.ascontiguousarray(
            ew2.reshape(NE, KO2, P, NG2, WG2, P).transpose(0, 3, 2, 4, 1, 5)
        ).astype(bf16),
        "iden": np.eye(P, dtype=np.float32),
    }
    xTs, xTbs, xh2s = [], [], []
    for c in range(NCORES):
        xt = np.ascontiguousarray(
            combined[c * T : (c + 1) * T].T.reshape(KO1, P, T).transpose(1, 0, 2)
        )
        xTs.append(xt)
        xh = np.ascontiguousarray(xt.astype(bf16))
        xTbs.append(xh)
        # chunk-major bf16 tokens for the gate: [P, TT, KO1, 512]
        xh2s.append(
            np.ascontiguousarray(
                xh.reshape(P, KO1, TT, 512).transpose(0, 2, 1, 3)
            )
        )
    return shared, xTs, xTbs, xh2s


def gate_inmaps(shared, xh2s):
    return [
        {
            "xh": xh2s[c],
            "gwh": shared["gwhf"],
            "gb1": shared["gb1"],
            "gw2b": shared["gw2b"],
            "gb2r": shared["gb2c"],
        }
        for c in range(NCORES)
    ]


_NC_CACHE = {}


def kernel_dense(**inputs):
    if "nc" not in _NC_CACHE:
        _NC_CACHE["nc"] = build_nc()
    nc = _NC_CACHE["nc"]

    shared, xTs, xTbs, _ = prep_inputs(inputs)
    in_maps = [
        {**shared, "xT": xTs[c], "xTb": xTbs[c]} for c in range(NCORES)
    ]
    res = run_bass_kernel_spmd(nc, in_maps, core_ids=list(range(NCORES)))
    outs = res.results

    fused = np.empty((N, E), np.float32)
    for c in range(NCORES):
        accT = outs[c]["accT"]  # [P, FO2, T]
        fused[c * T : (c + 1) * T] = accT.transpose(2, 1, 0).reshape(T, E)
    return fused


# ======================================================================
# Sparse (true MoE routing) two-phase path.
# ======================================================================


def build_nc_gate(reps=1):
    """Gate kernel: single bf16 pass of mm1 + bf16 mm2, fully pipelined
    per-fo-tile weight DMA and per-512-token-chunk activation DMA."""
    nc = bacc.Bacc("TRN2", target_bir_lowering=False, debug=False, num_devices=NCORES)
    xh = nc.dram_tensor("xh", [P, TT, KO1, 512], BF16, kind="ExternalInput").ap()
    gwh = nc.dram_tensor("gwh", [GFO, P, KO1, P], BF16, kind="ExternalInput").ap()
    gb1 = nc.dram_tensor("gb1", [P, GFO], F32, kind="ExternalInput").ap()
    gw2b = nc.dram_tensor("gw2b", [P, GFO, NE], BF16, kind="ExternalInput").ap()
    gb2r = nc.dram_tensor("gb2r", [NE, 1], F32, kind="ExternalInput").ap()
    lg = nc.dram_tensor("lg", [NE, T], F32, kind="ExternalOutput").ap()

    with tile.TileContext(nc) as tc:
        with (
            tc.tile_pool(name="sb", bufs=2) as sb,
            tc.tile_pool(name="gh", bufs=2) as ghp,
            tc.tile_pool(name="tmp", bufs=2) as tmp,
            tc.tile_pool(name="ps", bufs=2, space="PSUM") as ps,
            tc.tile_pool(name="psl", bufs=2, space="PSUM") as psl,
        ):
            for _rep in range(reps):
                gwh_s = sb.tile([P, GFO, KO1, P], BF16, tag="gwh")
                xh_s = sb.tile([P, TT, KO1, 512], BF16, tag="xh")
                gb1_s = sb.tile([P, GFO], F32, tag="gb1")
                gw2b_s = sb.tile([P, GFO, NE], BF16, tag="gw2b")
                gb2r_s = sb.tile([NE, 1], F32, tag="gb2r")
                # pipelined head: first fo-tile + first token chunk land first
                nc.sync.dma_start(gwh_s[:, 0], gwh[0])
                nc.scalar.dma_start(xh_s[:, 0], xh[:, 0])
                for fo in range(1, GFO):
                    nc.sync.dma_start(gwh_s[:, fo], gwh[fo])
                nc.sync.dma_start(gw2b_s[:], gw2b)
                nc.sync.dma_start(gb1_s[:], gb1)
                nc.sync.dma_start(gb2r_s[:], gb2r)
                nc.scalar.dma_start(xh_s[:, 1], xh[:, 1])

                for t2 in range(TT):
                    ghT = ghp.tile([P, GFO, 512], BF16, tag="ghT")
                    for fo in range(GFO):
                        pg = ps.tile([P, 512], F32, tag="pg")
                        for ko in range(KO1):
                            nc.tensor.matmul(
                                pg[:],
                                lhsT=gwh_s[:, fo, ko, :],
                                rhs=xh_s[:, t2, ko, :],
                                start=(ko == 0),
                                stop=(ko == KO1 - 1),
                            )
                        nc.scalar.activation(
                            ghT[:, fo, :], pg[:], GELU, bias=gb1_s[:, fo : fo + 1]
                        )
                    pl = psl.tile([NE, 512], F32, tag="pl")
                    for kc in range(GFO):
                        nc.tensor.matmul(
                            pl[:],
                            lhsT=gw2b_s[:, kc, :],
                            rhs=ghT[:, kc, :],
                            start=(kc == 0),
                            stop=(kc == GFO - 1),
                        )
                    lt = tmp.tile([NE, 512], F32, tag="lt")
                    nc.vector.tensor_scalar(lt[:], pl[:], gb2r_s[:], None, OP.add)
                    nc.scalar.dma_start(lg[:, t2 * 512 : (t2 + 1) * 512], lt[:])
    nc.compile()
    return nc


def build_nc_exp(SL, reps=1):
    """Expert-phase kernel.  SL is the per-core slot-capacity profile (list of
    ints, each a multiple of 8, <= S_CAP); every core runs the same program
    with slot j sized SL[j].  Weights stream per-slot in fo-groups."""
    SL = list(SL)
    EXP = len(SL)
    assert all(8 <= c <= S_CAP and c % 8 == 0 for c in SL)
    Tc = sum(SL)
    offs = [0]
    for c in SL:
        offs.append(offs[-1] + c)
    Smax = max(SL)
    nc = bacc.Bacc("TRN2", target_bir_lowering=False, debug=False, num_devices=NCORES)
    xTe = nc.dram_tensor("xTe", [P, KO1 * Tc], BF16, kind="ExternalInput").ap()
    wrow = nc.dram_tensor("wrow", [1, Tc], F32, kind="ExternalInput").ap()
    w1s = nc.dram_tensor(
        "w1s", [EXP, NG1, P, WG1, KO1, P], BF16, kind="ExternalInput"
    ).ap()
    b1s = nc.dram_tensor("b1s", [P, EXP, FO1], F32, kind="ExternalInput").ap()
    w2s = nc.dram_tensor(
        "w2s", [EXP, NG2, P, WG2, KO2, P], BF16, kind="ExternalInput"
    ).ap()
    b2s = nc.dram_tensor("b2s", [P, EXP, FO2], F32, kind="ExternalInput").ap()
    oT = nc.dram_tensor("oT", [P, FO2, Tc], F32, kind="ExternalOutput").ap()

    import contextlib

    with tile.TileContext(nc) as tc:
        with contextlib.ExitStack() as ctx:
            pers = ctx.enter_context(tc.tile_pool(name="pers", bufs=1))
            persb = ctx.enter_context(tc.tile_pool(name="persb", bufs=2))
            psA = ctx.enter_context(tc.tile_pool(name="psA", bufs=3, space="PSUM"))
            w1pool = ctx.enter_context(tc.tile_pool(name="w1p", bufs=2))
            w2pool = ctx.enter_context(tc.tile_pool(name="w2p", bufs=2))
            hpool = ctx.enter_context(tc.tile_pool(name="hp", bufs=1))
            spool = ctx.enter_context(tc.tile_pool(name="sp", bufs=2))
            opool = ctx.enter_context(tc.tile_pool(name="op", bufs=2))
            psB = ctx.enter_context(tc.tile_pool(name="psB", bufs=3, space="PSUM"))
            for _rep in range(reps):
                xTe_s = pers.tile([P, KO1 * Tc], BF16, tag="xTe")
                b1s_s = persb.tile([P, EXP, FO1], F32, tag="b1s")
                b2s_s = persb.tile([P, EXP, FO2], F32, tag="b2s")
                wb2 = pers.tile([P, Tc], F32, tag="wb2")

                # small head DMAs first so the PE isn't stalled behind the
                # bulk xTe transfer (same queue is FIFO)
                ones_sb = pers.tile([1, P], F32, tag="ones")
                nc.vector.memset(ones_sb[:], 1.0)
                wrow_s = persb.tile([1, Tc], F32, tag="wrow")
                nc.scalar.dma_start(wrow_s[:], wrow)
                nc.scalar.dma_start(b1s_s[:], b1s)
                nc.scalar.dma_start(b2s_s[:], b2s)
                # slot-major transfer order: slot j's tokens land long before
                # the PE reaches slot j
                for j in range(EXP):
                    nc.scalar.dma_start(
                        xTe_s[:, KO1 * offs[j] : KO1 * offs[j + 1]],
                        xTe[:, KO1 * offs[j] : KO1 * offs[j + 1]],
                    )
                for a, b in _chunks(Tc):
                    pw = psA.tile([P, 512], F32, tag="psA")
                    nc.tensor.matmul(
                        pw[:, : b - a],
                        lhsT=ones_sb[:],
                        rhs=wrow_s[:, a:b],
                        start=True,
                        stop=True,
                    )
                    nc.scalar.mul(wb2[:, a:b], pw[:, : b - a], 0.5)  # wb2 = w/2

                for j in range(EXP):
                    S = SL[j]
                    t0 = offs[j]
                    cks = _even_chunks(S)
                    hbig = hpool.tile([P, KO2 * Smax], BF16, tag="ht")
                    for g in range(NG1):
                        w1t = w1pool.tile([P, WG1, KO1, P], BF16, tag="w1t")
                        nc.sync.dma_start(w1t[:], w1s[j, g])
                        for q in range(WG1):
                            fo = WG1 * g + q
                            for a, b in cks:
                                pa = psA.tile([P, 512], F32, tag="psA")
                                for ko in range(KO1):
                                    base = KO1 * t0 + ko * S
                                    nc.tensor.matmul(
                                        pa[:, : b - a],
                                        lhsT=w1t[:, q, ko, :],
                                        rhs=xTe_s[:, base + a : base + b],
                                        start=(ko == 0),
                                        stop=(ko == KO1 - 1),
                                    )
                                nc.scalar.activation(
                                    hbig[:, fo * S + a : fo * S + b],
                                    pa[:, : b - a],
                                    GELU,
                                    bias=b1s_s[:, j, fo : fo + 1],
                                )
                    for g2 in range(NG2):
                        w2t = w2pool.tile([P, WG2, KO2, P], BF16, tag="w2t")
                        nc.sync.dma_start(w2t[:], w2s[j, g2])
                        for q in range(WG2):
                            fo2 = WG2 * g2 + q
                            obuf = opool.tile([P, Smax], F32, tag="obuf")
                            for a, b in cks:
                                pb = psB.tile([P, 512], F32, tag="psB")
                                for ko in range(KO2):
                                    nc.tensor.matmul(
                                        pb[:, : b - a],
                                        lhsT=w2t[:, q, ko, :],
                                        rhs=hbig[:, ko * S + a : ko * S + b],
                                        start=(ko == 0),
                                        stop=(ko == KO2 - 1),
                                    )
                                st = spool.tile([P, 512], F32, tag="st")
                                # tanh(0.5*o + 0.5*b2)  (b2s input is pre-halved)
                                nc.scalar.activation(
                                    st[:, : b - a],
                                    pb[:, : b - a],
                                    AF.Tanh,
                                    bias=b2s_s[:, j, fo2 : fo2 + 1],
                                    scale=0.5,
                                )
                                # out = w*sigmoid(o) = wb2 + wb2*tanh
                                nc.vector.tensor_tensor(
                                    obuf[:, a:b],
                                    st[:, : b - a],
                                    wb2[:, t0 + a : t0 + b],
                                    OP.mult,
                                )
                                nc.vector.tensor_tensor(
                                    obuf[:, a:b],
                                    obuf[:, a:b],
                                    wb2[:, t0 + a : t0 + b],
                                    OP.add,
                                )
                            nc.gpsimd.dma_start(
                                oT[:, fo2, t0 : t0 + S], obuf[:, :S]
                            )
    nc.compile()
    return nc


def _host_exact_logits(inputs, rows):
    """Exact fp32 gate logits for the given token rows."""
    c = np.asarray(inputs["combined"], np.float32)[rows]
    gh = _gelu32(c @ np.asarray(inputs["gate_w1"], np.float32)
                 + np.asarray(inputs["gate_b1"], np.float32))
    return (gh @ np.asarray(inputs["gate_w2"], np.float32)
            + np.asarray(inputs["gate_b2"], np.float32))


# ---------- slot-profile selection: minimize PE cost under bin packing ----
#
# Cost model (per core, ns): each token costs 432 PE cycles = 180 ns, and
# each (weight-pass x token-chunk) costs a flat ~48 ns of serialized
# LDWEIGHTS in the self-loading matmul => 432*48 ~ 20.7 us per slot-chunk.
# So J(SL) = 180*sum(SL) + 20700*sum(ceil(SL_j/512)), minimized subject to
# the experts packing into 8 copies of SL with <=1 expert per bin.

_PROFILE_SEEDS = [(512, 488, 480, 424, 168)]  # known-good for the target dist


def _bundles(c, SL):
    """Pareto-minimal bin bundles (per-size counts) covering c tokens."""
    import itertools

    E = len(SL)
    rng = [range(0, min(NCORES, -(-c // SL[j])) + 1) for j in range(E)]
    combos = []
    for n in itertools.product(*rng):
        if sum(n) <= NCORES and sum(nj * s for nj, s in zip(n, SL)) >= c:
            combos.append(n)
    combos.sort(key=sum)
    keep = []
    for n in combos:
        if not any(all(m[j] <= n[j] for j in range(E)) and m != n for m in keep):
            keep.append(n)
    return keep


def _pack(cnt, SL):
    """Exact bin-cover DP.  Returns per-expert bundles (counts per size
    class) or None if the experts don't fit 8 copies of SL."""
    E = len(SL)
    if E > 6:
        return None
    order = np.argsort(-np.asarray(cnt))
    dims = (NCORES + 1,) * E
    nstates = int(np.prod(dims))
    reach = np.zeros(nstates, bool)
    reach[np.ravel_multi_index((NCORES,) * E, dims)] = True
    BL, layers = [], [reach]
    for e in order:
        B = _bundles(int(cnt[e]), SL)
        if not B:
            return None
        BL.append(B)
        idx = np.nonzero(reach)[0]
        if not len(idx):
            return None
        coords = np.stack(np.unravel_index(idx, dims), 1)
        new = np.zeros(nstates, bool)
        for b in B:
            c2 = coords - b
            ok = (c2 >= 0).all(1)
            if ok.any():
                new[np.ravel_multi_index(c2[ok].T.tolist(), dims)] = True
        reach = new
        layers.append(reach)
    if not reach.any():
        return None
    fin = np.nonzero(reach)[0][0]
    state = np.array(np.unravel_index(fin, dims))
    rev = []
    for i in range(len(order) - 1, -1, -1):
        found = None
        for b in BL[i]:
            p = state + b
            if (p <= NCORES).all() and layers[i][np.ravel_multi_index(p.tolist(), dims)]:
                found = b
                state = p
                break
        assert found is not None
        rev.append(found)
    rev.reverse()
    asg = [None] * NE
    for i, e in enumerate(order):
        asg[int(e)] = rev[i]
    return asg


def _waterfill_profile(cnt, EXP):
    """Near-equal waterfill pieces -> sorted groups of 8 -> profile + direct
    assignment (always feasible)."""
    k = np.ones(NE, np.int64)
    for _ in range(NCORES * EXP - NE):
        j = np.argmax(-(-cnt // k))
        k[j] += 1
    pieces = []
    for e in range(NE):
        base, rem = divmod(int(cnt[e]), int(k[e]))
        for i in range(int(k[e])):
            pieces.append((base + (1 if i < rem else 0), e))
    pieces.sort(key=lambda t: -t[0])
    SL = tuple(
        max(8, -(-max(pieces[j * NCORES + c][0] for c in range(NCORES)) // 8) * 8)
        for j in range(EXP)
    )
    asg = [[0] * EXP for _ in range(NE)]
    for j in range(EXP):
        for c in range(NCORES):
            asg[pieces[j * NCORES + c][1]][j] += 1
    return SL, asg


def _profile_cost(SL):
    return 180 * sum(SL) + 20700 * sum(-(-s // 512) for s in SL)


def choose_profile(cnt):
    """Pick the cheapest feasible slot profile for the given expert loads."""
    best = None
    for SL in _PROFILE_SEEDS:
        if max(SL) > S_CAP:
            continue
        asg = _pack(cnt, SL)
        if asg is not None:
            j = _profile_cost(SL)
            if best is None or j < best[0]:
                best = (j, tuple(SL), asg)
    for EXP in (5, 4, 6, 8):
        SL, asg = _waterfill_profile(cnt, EXP)
        if max(SL) > S_CAP:
            continue
        j = _profile_cost(SL)
        if best is None or j < best[0]:
            best = (j, SL, asg)
    if best is None:
        return None, None
    return list(best[1]), best[2]


def route(logits):
    """Host softmax/top-2/normalize + slot assignment via profile packing.

    Returns (slots, SL): slots[c][j] = (expert, token_idx[SL_j], weight[SL_j])
    padded with token 0 / weight 0."""
    lg = logits.astype(np.float32)
    m = lg.max(axis=1, keepdims=True)
    p = np.exp(lg - m)
    p /= p.sum(axis=1, keepdims=True)
    order = np.argsort(-p, axis=1, kind="stable")
    i1, i2 = order[:, 0], order[:, 1]
    r = np.arange(lg.shape[0])
    w1 = p[r, i1]
    w2 = p[r, i2]
    s = w1 + w2
    w1, w2 = w1 / s, w2 / s

    toks, wts = [], []
    for e in range(NE):
        t1 = np.nonzero(i1 == e)[0]
        t2 = np.nonzero(i2 == e)[0]
        toks.append(np.concatenate([t1, t2]))
        wts.append(np.concatenate([w1[t1], w2[t2]]).astype(np.float32))
    cnt = np.array([len(t) for t in toks])

    SL, asg = choose_profile(cnt)
    if SL is None:
        return None, None
    EXP = len(SL)
    # instantiate bins: class j has 8 instances (one per core), round-robin
    nxt = [0] * EXP  # next free core per class
    slots = [[None] * EXP for _ in range(NCORES)]
    for e in range(NE):
        pos = 0
        # fill this expert's bins largest-class-first so the partial bin is
        # the smallest one
        for j in range(EXP):
            for _ in range(asg[e][j]):
                c = nxt[j]
                nxt[j] += 1
                take = max(0, min(SL[j], cnt[e] - pos))
                tt = np.zeros(SL[j], np.int64)
                ww = np.zeros(SL[j], np.float32)
                tt[:take] = toks[e][pos : pos + take]
                ww[:take] = wts[e][pos : pos + take]
                slots[c][j] = (e, tt, ww)
                pos += take
        assert pos >= cnt[e], f"expert {e}: {pos} < {cnt[e]}"
    # unused bins -> padding (expert 0, zero weights)
    for j in range(EXP):
        for c in range(nxt[j], NCORES):
            slots[c][j] = (
                0,
                np.zeros(SL[j], np.int64),
                np.zeros(SL[j], np.float32),
            )
    return slots, SL


def build_emaps(inputs, shared, slots, SL):
    bf16 = mybir.dt.np(BF16)
    EXP = len(SL)
    Tc = sum(SL)
    combined = np.asarray(inputs["combined"], np.float32)
    emaps = []
    for c in range(NCORES):
        eids = [slots[c][j][0] for j in range(EXP)]
        ws = np.concatenate([slots[c][j][2] for j in range(EXP)])
        blocks = []
        for j in range(EXP):
            xg = combined[slots[c][j][1]]  # [SL[j], D]
            blocks.append(
                xg.T.reshape(KO1, P, SL[j]).transpose(1, 0, 2).reshape(P, KO1 * SL[j])
            )
        emaps.append(
            {
                "xTe": np.ascontiguousarray(np.concatenate(blocks, axis=1)).astype(
                    bf16
                ),
                "wrow": ws.reshape(1, Tc).astype(np.float32),
                "w1s": np.ascontiguousarray(shared["w1g"][eids]),
                "b1s": np.ascontiguousarray(shared["b1e"][:, eids, :]),
                "w2s": np.ascontiguousarray(shared["w2g"][eids]),
                "b2s": np.ascontiguousarray(shared["b2e"][:, eids, :]),
            }
        )
    return emaps


def kernel_sparse(**inputs):
    shared, xTs, _, xh2s = prep_inputs(inputs)

    if "gate" not in _NC_CACHE:
        _NC_CACHE["gate"] = build_nc_gate()
    ncg = _NC_CACHE["gate"]
    gmaps = gate_inmaps(shared, xh2s)
    gres = run_bass_kernel_spmd(ncg, gmaps, core_ids=list(range(NCORES)))
    logits = np.concatenate(
        [gres.results[c]["lg"].T for c in range(NCORES)]
    )

    # host fixup: recompute ambiguous tokens (top2/3 gap under THETA) exactly
    srt = np.sort(logits, axis=1)
    amb = np.nonzero((srt[:, -2] - srt[:, -3]) < THETA)[0]
    if len(amb):
        logits[amb] = _host_exact_logits(inputs, amb)

    slots, SL = route(logits)
    if SL is None or max(SL) > S_CAP or sum(SL) > 3000:  # pathological routing
        return kernel_dense(**inputs)
    EXP = len(SL)
    Tc = sum(SL)
    offs = [0]
    for ccap in SL:
        offs.append(offs[-1] + ccap)

    key = ("exp", tuple(SL))
    if key not in _NC_CACHE:
        _NC_CACHE[key] = build_nc_exp(SL)
    nce = _NC_CACHE[key]

    emaps = build_emaps(inputs, shared, slots, SL)
    _NC_CACHE["last_emaps"] = emaps
    _NC_CACHE["last_SL"] = SL
    eres = run_bass_kernel_spmd(nce, emaps, core_ids=list(range(NCORES)))

    fused = np.zeros((N, E), np.float32)
    for c in range(NCORES):
        rows = eres.results[c]["oT"].transpose(2, 1, 0).reshape(Tc, E)
        for j in range(EXP):
            # np.add.at: padding reuses token 0 with an all-zero weight row
            np.add.at(fused, slots[c][j][1], rows[offs[j] : offs[j + 1]])
    return fused


MODE = "sparse"


def kernel(**inputs):
    if MODE == "sparse":
        try:
            return kernel_sparse(**inputs)
        except Exception:
            return kernel_dense(**inputs)
    return kernel_dense(**inputs)


if __name__ == "__main__":  # dev smoke test only; harness imports kernel()
    import reference  # noqa: PLC0415 -- not needed when imported as a module

    inputs = {k: np.asarray(v) for k, v in reference.setup_inputs().items()}
    out = kernel(**inputs)
    print(out.shape, out.dtype)


# revision 14
# speedup vs baseline: 1.0544x; 1.0331x over previous
"""MoE fusion kernel for Trainium2 (8 NeuronCores, two-phase sparse routing).

Structure
---------
Phase 1 (gate kernel, data-parallel over tokens): each core computes gate
logits for its 1024 tokens with a SINGLE bf16 pass of the big matmul
(max logit error ~7e-3), fully pipelined per-fo-tile weight DMA and
per-512-token-chunk moving-operand DMA, bf16 second matmul.  Tokens whose
top2/top3 logit gap is below THETA=2.5e-2 (~800 of 8192) are ambiguous at
that precision; the host recomputes their logits exactly (fp32, ~1 GMAC --
same spirit as the host softmax/top-k).  All other tokens route correctly
because their gap exceeds the worst-case device error by >1.7x; their
combine weights use the device logits (weight error <4e-3 -> output error
<2e-3).  ~42 us/pass on HW.

Host: softmax/top-2/renormalize, then slot assignment by exact bin-cover
DP.  Cost model: each token costs 432 PE cycles (=180 ns) and each
(weight-pass x token-chunk) costs ~48 ns of LDWEIGHTS serialized into the
self-loading matmul (measured; the PE reloads the stationary operand every
matmul), i.e. ~20.7 us per slot-chunk.  choose_profile() minimizes
J = 180*sum(SL) + 20700*sum(ceil(SL_j/512)) over slot-capacity profiles SL
subject to the 12 experts packing into 8 copies of SL with at most one
expert per slot (exact DP).  For the target load distribution this gives
SL=[512, 488, 480, 424, 168] (5 slots/core, 1.2% padding, 5 chunks).
Slots load each expert's weights once (14.2 MB/slot); 5 slots/core keeps
the weight stream (~200 us/core) well under the PE time (~475 us/core).

Phase 2 (expert kernel): each core runs its slots; per slot one expert's
weights stream in 1.6 MB fo-groups (6 for W1, 3 for W2, sync queue; bulk
activations on the gpsimd queue) and the MLP runs in bf16, feature-major,
weights stationary:

    x.T [1536, S] -> h.T = gelu(W1.T x.T) [3072, S] -> o.T = W2.T h.T [768, S]

Token chunks are even splits of S into ceil(S/512) pieces (PSUM bank
limit).  The final sigmoid folds into the combine weights as
out = w*sigmoid(o) = wb2 + wb2*tanh(0.5*o + 0.5*b2), wb2 = w/2, so only
the one 'gelu_and_others' ACT table set (gelu + tanh + copy) is ever
loaded.  The host scatter-adds the pre-weighted slot outputs into the
[N, E] result.  ~495 us/pass on HW.

Measurement note: per-pass times come from rep-NEFFs (body repeated R
times); R must be large enough that one dispatch carries >>3 ms of device
work or the burst measurement reads client dispatch cost instead of
device time (gate uses R=128).

A dense all-experts fallback kernel handles pathologically skewed routing.
"""

import math

import numpy as np

try:
    import concourse  # noqa: F401
except ImportError:  # pragma: no cover
    import sys

    sys.path.insert(0, "/opt/trn_rl_repo")

import concourse.bass as bass  # noqa: F401
import concourse.mybir as mybir
import concourse.tile as tile
from concourse import bacc
from concourse.bass_utils import run_bass_kernel_spmd

# Problem shapes (hardcoded per contest rules).
N, D, E, H, NE = 8192, 1536, 768, 3072, 12
NCORES = 8
T = N // NCORES  # 1024 tokens per core
P = 128
KO1 = D // P  # 12   k-tiles of the first expert matmul
FO1 = H // P  # 24   feature-tiles of h
KO2 = H // P  # 24   k-tiles of the second expert matmul
FO2 = E // P  # 6    feature-tiles of the output
GFO = E // P  # 6    feature-tiles of the gate hidden
TT = T // 512  # 2   512-token chunks of the gate moving operand

F32 = mybir.dt.float32
BF16 = mybir.dt.bfloat16
AF = mybir.ActivationFunctionType
OP = mybir.AluOpType

GELU = AF.Gelu  # test.py sim-mode substitutes Tanh (CoreSim lacks Gelu)

WG1 = 4  # fo-tiles per W1 DMA group (FO1/WG1 = 6 groups)
WG2 = 2  # fo2-tiles per W2 DMA group (FO2/WG2 = 3 groups)
NG1 = FO1 // WG1
NG2 = FO2 // WG2
THETA = 2.5e-2  # top2/3 gap below which the host recomputes exact logits
S_CAP = 1024  # beyond this the phase-2 working set won't fit SBUF -> dense


def _chunks(total, step=512):
    return [(a, min(a + step, total)) for a in range(0, total, step)]


def _even_chunks(S):
    """Split S into ceil(S/512) near-equal chunks (multiples of 8)."""
    k = -(-S // 512)
    out, a = [], 0
    for i in range(k):
        w = ((S - a) // (k - i) + 7) // 8 * 8 if i < k - 1 else S - a
        w = min(w, 512, S - a)
        out.append((a, a + w))
        a += w
    return out


def _erf(x):
    try:
        from scipy.special import erf as _serf  # noqa: PLC0415

        return _serf(x)
    except ImportError:
        u = np.frompyfunc(math.erf, 1, 1)
        return u(x.astype(np.float64)).astype(np.float32)


def _gelu32(x):
    x = x.astype(np.float32)
    return (0.5 * x * (1.0 + _erf(x / np.sqrt(np.float32(2.0))))).astype(np.float32)


# ======================================================================
# Dense all-experts fallback (unchanged from the robust baseline).
# ======================================================================


def _emit_dense(tc, aps):
    nc = tc.nc
    (xT, xTb, gw1, gb1, gw2, gb2r, w1e, b1e, w2e, b2e, iden, out) = aps

    import contextlib

    with contextlib.ExitStack() as ctx:
        pers = ctx.enter_context(tc.tile_pool(name="pers", bufs=1))
        xTb_s = pers.tile([P, KO1, T], BF16)
        nc.sync.dma_start(xTb_s[:], xTb)
        b1e_s = pers.tile([P, NE, FO1], F32)
        nc.sync.dma_start(b1e_s[:], b1e)
        b2e_s = pers.tile([P, NE, FO2], F32)
        nc.sync.dma_start(b2e_s[:], b2e)
        acc = pers.tile([P, FO2, T], F32)
        wT = pers.tile([NE, T], F32)
        ones_sb = pers.tile([1, P], F32)
        nc.vector.memset(ones_sb[:], 1.0)

        with (
            tc.tile_pool(name="gate_sb", bufs=1) as gsb,
            tc.tile_pool(name="gate_tmp", bufs=2) as gtmp,
            tc.tile_pool(name="gate_ps", bufs=2, space="PSUM") as gps,
            tc.tile_pool(name="gate_ps_small", bufs=2, space="PSUM") as gpss,
        ):
            xT_s = gsb.tile([P, KO1, T], F32)
            nc.sync.dma_start(xT_s[:], xT)
            gw1_s = gsb.tile([P, KO1, E], F32)
            nc.sync.dma_start(gw1_s[:], gw1)
            gb1_s = gsb.tile([P, GFO], F32)
            nc.sync.dma_start(gb1_s[:], gb1)
            gw2_s = gsb.tile([P, GFO, NE], F32)
            nc.sync.dma_start(gw2_s[:], gw2)
            gb2r_s = gsb.tile([P, NE], F32)
            nc.sync.dma_start(gb2r_s[:], gb2r)
            iden_s = gsb.tile([P, P], F32)
            nc.sync.dma_start(iden_s[:], iden)
            ghT = gsb.tile([P, GFO, T], F32)

            for fo in range(GFO):
                pg = gps.tile([P, T], F32, tag="gps")
                for t2 in range(TT):
                    for ko in range(KO1):
                        nc.tensor.matmul(
                            pg[:, t2 * 512 : (t2 + 1) * 512],
                            lhsT=gw1_s[:, ko, fo * P : (fo + 1) * P],
                            rhs=xT_s[:, ko, t2 * 512 : (t2 + 1) * 512],
                            start=(ko == 0),
                            stop=(ko == KO1 - 1),
                        )
                nc.scalar.activation(
                    ghT[:, fo, :], pg[:], GELU, bias=gb1_s[:, fo : fo + 1]
                )

            for tt in range(T // P):
                pl = gpss.tile([P, NE], F32, tag="gpl")
                for fo in range(GFO):
                    nc.tensor.matmul(
                        pl[:],
                        lhsT=ghT[:, fo, tt * P : (tt + 1) * P],
                        rhs=gw2_s[:, fo, :],
                        start=(fo == 0),
                        stop=(fo == GFO - 1),
                    )
                lt = gtmp.tile([P, NE], F32, tag="lt")
                nc.vector.tensor_tensor(lt[:], pl[:], gb2r_s[:], OP.add)
                m8 = gtmp.tile([P, 8], F32, tag="m8")
                nc.vector.max(m8[:], lt[:])
                dlt = gtmp.tile([P, 1], F32, tag="dlt")
                nc.vector.tensor_tensor(dlt[:], m8[:, 0:1], m8[:, 1:2], OP.subtract)
                w1v = gtmp.tile([P, 1], F32, tag="w1v")
                nc.scalar.activation(w1v[:], dlt[:], AF.Tanh, scale=0.5)
                nc.vector.tensor_scalar(w1v[:], w1v[:], 0.5, 0.5, OP.mult, OP.add)
                w2v = gtmp.tile([P, 1], F32, tag="w2v")
                nc.vector.tensor_scalar(w2v[:], w1v[:], -1.0, 1.0, OP.mult, OP.add)
                eq1 = gtmp.tile([P, NE], F32, tag="eq1")
                nc.vector.tensor_scalar(eq1[:], lt[:], m8[:, 0:1], None, OP.is_equal)
                nc.vector.tensor_scalar(eq1[:], eq1[:], w1v[:], None, OP.mult)
                eq2 = gtmp.tile([P, NE], F32, tag="eq2")
                nc.vector.tensor_scalar(eq2[:], lt[:], m8[:, 1:2], None, OP.is_equal)
                nc.vector.tensor_scalar(eq2[:], eq2[:], w2v[:], None, OP.mult)
                nc.vector.tensor_tensor(eq1[:], eq1[:], eq2[:], OP.add)
                ptw = gpss.tile([NE, P], F32, tag="gpt")
                nc.tensor.transpose(ptw[:], eq1[:], iden_s[:])
                nc.vector.tensor_copy(wT[:, tt * P : (tt + 1) * P], ptw[:])

        w1pool = ctx.enter_context(tc.tile_pool(name="w1p", bufs=2))
        w2pool = ctx.enter_context(tc.tile_pool(name="w2p", bufs=3))
        hpool = ctx.enter_context(tc.tile_pool(name="hp", bufs=FO1 + 4))
        wbpool = ctx.enter_context(tc.tile_pool(name="wbp", bufs=2))
        spool = ctx.enter_context(tc.tile_pool(name="sp", bufs=2))
        tpool = ctx.enter_context(tc.tile_pool(name="tp", bufs=2))
        psA = ctx.enter_context(tc.tile_pool(name="psA", bufs=3, space="PSUM"))
        psB = ctx.enter_context(tc.tile_pool(name="psB", bufs=3, space="PSUM"))

        for e in range(NE):
            wb = wbpool.tile([P, T], F32, tag="wb")
            wrow = wbpool.tile([1, T], F32, tag="wrow")
            nc.sync.dma_start(wrow[:], wT[e : e + 1, :])
            pwb = psA.tile([P, T], F32, tag="psA")
            for t2 in range(TT):
                nc.tensor.matmul(
                    pwb[:, t2 * 512 : (t2 + 1) * 512],
                    lhsT=ones_sb[:],
                    rhs=wrow[:, t2 * 512 : (t2 + 1) * 512],
                    start=True,
                    stop=True,
                )
            nc.vector.tensor_copy(wb[:], pwb[:])

            hts = []
            for fop in range(FO1 // 2):
                w1t = w1pool.tile([P, 2, KO1, P], BF16, tag="w1t")
                nc.sync.dma_start(w1t[:], w1e[e, fop])
                for q in range(2):
                    fo = 2 * fop + q
                    pa = psA.tile([P, T], F32, tag="psA")
                    for ko in range(KO1):
                        for t2 in range(TT):
                            nc.tensor.matmul(
                                pa[:, t2 * 512 : (t2 + 1) * 512],
                                lhsT=w1t[:, q, ko, :],
                                rhs=xTb_s[:, ko, t2 * 512 : (t2 + 1) * 512],
                                start=(ko == 0),
                                stop=(ko == KO1 - 1),
                            )
                    ht = hpool.tile([P, T], BF16, tag="ht")
                    nc.scalar.activation(
                        ht[:], pa[:], GELU, bias=b1e_s[:, e, fo : fo + 1]
                    )
                    hts.append(ht)

            for fop2 in range(FO2 // 2):
                w2t = w2pool.tile([P, 2, KO2, P], BF16, tag="w2t")
                nc.sync.dma_start(w2t[:], w2e[e, fop2])
                for q2 in range(2):
                    fo2 = 2 * fop2 + q2
                    pb = psB.tile([P, T], F32, tag="psB")
                    for ko in range(KO2):
                        for t2 in range(TT):
                            nc.tensor.matmul(
                                pb[:, t2 * 512 : (t2 + 1) * 512],
                                lhsT=w2t[:, q2, ko, :],
                                rhs=hts[ko][:, t2 * 512 : (t2 + 1) * 512],
                                start=(ko == 0),
                                stop=(ko == KO2 - 1),
                            )
                    st = spool.tile([P, T], F32, tag="st")
                    nc.scalar.activation(
                        st[:], pb[:], AF.Tanh, bias=b2e_s[:, e, fo2 : fo2 + 1], scale=0.5
                    )
                    if e == 0:
                        nc.vector.tensor_tensor(acc[:, fo2, :], st[:], wb[:], OP.mult)
                    else:
                        tmp = tpool.tile([P, T], F32, tag="tmp")
                        nc.vector.tensor_tensor(tmp[:], st[:], wb[:], OP.mult)
                        nc.vector.tensor_tensor(
                            acc[:, fo2, :], acc[:, fo2, :], tmp[:], OP.add
                        )

        for fo2 in range(FO2):
            fin = tpool.tile([P, T], F32, tag="fin")
            nc.vector.tensor_scalar(fin[:], acc[:, fo2, :], 0.5, 0.5, OP.mult, OP.add)
            nc.sync.dma_start(out[:, fo2, :], fin[:])


def build_nc():
    nc = bacc.Bacc(
        "TRN2", target_bir_lowering=False, debug=False, num_devices=NCORES
    )
    aps = (
        nc.dram_tensor("xT", [P, KO1, T], F32, kind="ExternalInput").ap(),
        nc.dram_tensor("xTb", [P, KO1, T], BF16, kind="ExternalInput").ap(),
        nc.dram_tensor("gw1", [P, KO1, E], F32, kind="ExternalInput").ap(),
        nc.dram_tensor("gb1", [P, GFO], F32, kind="ExternalInput").ap(),
        nc.dram_tensor("gw2", [P, GFO, NE], F32, kind="ExternalInput").ap(),
        nc.dram_tensor("gb2r", [P, NE], F32, kind="ExternalInput").ap(),
        nc.dram_tensor(
            "w1e", [NE, FO1 // 2, P, 2, KO1, P], BF16, kind="ExternalInput"
        ).ap(),
        nc.dram_tensor("b1e", [P, NE, FO1], F32, kind="ExternalInput").ap(),
        nc.dram_tensor(
            "w2e", [NE, FO2 // 2, P, 2, KO2, P], BF16, kind="ExternalInput"
        ).ap(),
        nc.dram_tensor("b2e", [P, NE, FO2], F32, kind="ExternalInput").ap(),
        nc.dram_tensor("iden", [P, P], F32, kind="ExternalInput").ap(),
        nc.dram_tensor("accT", [P, FO2, T], F32, kind="ExternalOutput").ap(),
    )
    with tile.TileContext(nc) as tc:
        _emit_dense(tc, aps)
    nc.compile()
    return nc


def prep_inputs(inputs):
    """Host-side sharding / relayout shared by dense + sparse paths."""
    bf16 = mybir.dt.np(BF16)
    combined = np.asarray(inputs["combined"], np.float32)
    gate_w1 = np.asarray(inputs["gate_w1"], np.float32)
    gate_b1 = np.asarray(inputs["gate_b1"], np.float32)
    gate_w2 = np.asarray(inputs["gate_w2"], np.float32)
    gate_b2 = np.asarray(inputs["gate_b2"], np.float32)
    ew1 = np.asarray(inputs["ew1"], np.float32)
    eb1 = np.asarray(inputs["eb1"], np.float32)
    ew2 = np.asarray(inputs["ew2"], np.float32)
    eb2 = np.asarray(inputs["eb2"], np.float32)

    gw1r = np.ascontiguousarray(gate_w1.reshape(KO1, P, E).transpose(1, 0, 2))
    # fo-major bf16 gate W1 for the pipelined per-fo-tile DMA
    gwhf = np.ascontiguousarray(
        gate_w1.reshape(KO1, P, GFO, P).transpose(2, 1, 0, 3)
    ).astype(bf16)
    shared = {
        "gw1": gw1r,
        "gwhf": gwhf,
        "gb1": np.ascontiguousarray(gate_b1.reshape(GFO, P).T),
        "gw2": np.ascontiguousarray(gate_w2.reshape(GFO, P, NE).transpose(1, 0, 2)),
        "gw2b": np.ascontiguousarray(
            gate_w2.reshape(GFO, P, NE).transpose(1, 0, 2)
        ).astype(bf16),
        "gb2r": np.ascontiguousarray(np.broadcast_to(gate_b2, (P, NE))),
        "gb2c": np.ascontiguousarray(gate_b2.reshape(NE, 1)),
        # dense-path fo-pair layouts
        "w1e": np.ascontiguousarray(
            ew1.reshape(NE, KO1, P, FO1 // 2, 2, P).transpose(0, 3, 2, 4, 1, 5)
        ).astype(bf16),
        "b1e": np.ascontiguousarray(eb1.reshape(NE, FO1, P).transpose(2, 0, 1)),
        "w2e": np.ascontiguousarray(
            ew2.reshape(NE, KO2, P, FO2 // 2, 2, P).transpose(0, 3, 2, 4, 1, 5)
        ).astype(bf16),
        "b2e": np.ascontiguousarray(
            (0.5 * eb2).reshape(NE, FO2, P).transpose(2, 0, 1)
        ),
        # sparse-path fo-group layouts
        "w1g": np.ascontiguousarray(
            ew1.reshape(NE, KO1, P, NG1, WG1, P).transpose(0, 3, 2, 4, 1, 5)
        ).astype(bf16),
        "w2g": np.ascontiguousarray(
            ew2.reshape(NE, KO2, P, NG2, WG2, P).transpose(0, 3, 2, 4, 1, 5)
        ).astype(bf16),
        "iden": np.eye(P, dtype=np.float32),
    }
    xTs, xTbs, xh2s = [], [], []
    for c in range(NCORES):
        xt = np.ascontiguousarray(
            combined[c * T : (c + 1) * T].T.reshape(KO1, P, T).transpose(1, 0, 2)
        )
        xTs.append(xt)
        xh = np.ascontiguousarray(xt.astype(bf16))
        xTbs.append(xh)
        # chunk-major bf16 tokens for the gate: [P, TT, KO1, 512]
        xh2s.append(
            np.ascontiguousarray(
                xh.reshape(P, KO1, TT, 512).transpose(0, 2, 1, 3)
            )
        )
    return shared, xTs, xTbs, xh2s


def gate_inmaps(shared, xh2s):
    return [
        {
            "xh": xh2s[c],
            "gwh": shared["gwhf"],
            "gb1": shared["gb1"],
            "gw2b": shared["gw2b"],
            "gb2r": shared["gb2c"],
        }
        for c in range(NCORES)
    ]


_NC_CACHE = {}


def kernel_dense(**inputs):
    if "nc" not in _NC_CACHE:
        _NC_CACHE["nc"] = build_nc()
    nc = _NC_CACHE["nc"]

    shared, xTs, xTbs, _ = prep_inputs(inputs)
    in_maps = [
        {**shared, "xT": xTs[c], "xTb": xTbs[c]} for c in range(NCORES)
    ]
    res = run_bass_kernel_spmd(nc, in_maps, core_ids=list(range(NCORES)))
    outs = res.results

    fused = np.empty((N, E), np.float32)
    for c in range(NCORES):
        accT = outs[c]["accT"]  # [P, FO2, T]
        fused[c * T : (c + 1) * T] = accT.transpose(2, 1, 0).reshape(T, E)
    return fused


# ======================================================================
# Sparse (true MoE routing) two-phase path.
# ======================================================================


def build_nc_gate(reps=1):
    """Gate kernel: single bf16 pass of mm1 + bf16 mm2, fully pipelined
    per-fo-tile weight DMA and per-512-token-chunk activation DMA."""
    nc = bacc.Bacc("TRN2", target_bir_lowering=False, debug=False, num_devices=NCORES)
    xh = nc.dram_tensor("xh", [P, TT, KO1, 512], BF16, kind="ExternalInput").ap()
    gwh = nc.dram_tensor("gwh", [GFO, P, KO1, P], BF16, kind="ExternalInput").ap()
    gb1 = nc.dram_tensor("gb1", [P, GFO], F32, kind="ExternalInput").ap()
    gw2b = nc.dram_tensor("gw2b", [P, GFO, NE], BF16, kind="ExternalInput").ap()
    gb2r = nc.dram_tensor("gb2r", [NE, 1], F32, kind="ExternalInput").ap()
    lg = nc.dram_tensor("lg", [NE, T], F32, kind="ExternalOutput").ap()

    with tile.TileContext(nc) as tc:
        with (
            tc.tile_pool(name="sb", bufs=2) as sb,
            tc.tile_pool(name="gh", bufs=2) as ghp,
            tc.tile_pool(name="tmp", bufs=2) as tmp,
            tc.tile_pool(name="ps", bufs=2, space="PSUM") as ps,
            tc.tile_pool(name="psl", bufs=2, space="PSUM") as psl,
        ):
            for _rep in range(reps):
                gwh_s = sb.tile([P, GFO, KO1, P], BF16, tag="gwh")
                xh_s = sb.tile([P, TT, KO1, 512], BF16, tag="xh")
                gb1_s = sb.tile([P, GFO], F32, tag="gb1")
                gw2b_s = sb.tile([P, GFO, NE], BF16, tag="gw2b")
                gb2r_s = sb.tile([NE, 1], F32, tag="gb2r")
                # pipelined head: first fo-tile + first token chunk land first
                nc.sync.dma_start(gwh_s[:, 0], gwh[0])
                nc.scalar.dma_start(xh_s[:, 0], xh[:, 0])
                for fo in range(1, GFO):
                    nc.sync.dma_start(gwh_s[:, fo], gwh[fo])
                nc.sync.dma_start(gw2b_s[:], gw2b)
                nc.sync.dma_start(gb1_s[:], gb1)
                nc.sync.dma_start(gb2r_s[:], gb2r)
                nc.scalar.dma_start(xh_s[:, 1], xh[:, 1])

                for t2 in range(TT):
                    ghT = ghp.tile([P, GFO, 512], BF16, tag="ghT")
                    for fo in range(GFO):
                        pg = ps.tile([P, 512], F32, tag="pg")
                        for ko in range(KO1):
                            nc.tensor.matmul(
                                pg[:],
                                lhsT=gwh_s[:, fo, ko, :],
                                rhs=xh_s[:, t2, ko, :],
                                start=(ko == 0),
                                stop=(ko == KO1 - 1),
                            )
                        nc.scalar.activation(
                            ghT[:, fo, :], pg[:], GELU, bias=gb1_s[:, fo : fo + 1]
                        )
                    pl = psl.tile([NE, 512], F32, tag="pl")
                    for kc in range(GFO):
                        nc.tensor.matmul(
                            pl[:],
                            lhsT=gw2b_s[:, kc, :],
                            rhs=ghT[:, kc, :],
                            start=(kc == 0),
                            stop=(kc == GFO - 1),
                        )
                    lt = tmp.tile([NE, 512], F32, tag="lt")
                    nc.vector.tensor_scalar(lt[:], pl[:], gb2r_s[:], None, OP.add)
                    nc.scalar.dma_start(lg[:, t2 * 512 : (t2 + 1) * 512], lt[:])
    nc.compile()
    return nc


def build_nc_exp(SL, reps=1):
    """Expert-phase kernel.  SL is the per-core slot-capacity profile (list of
    ints, each a multiple of 8, <= S_CAP); every core runs the same program
    with slot j sized SL[j].  Weights stream per-slot in fo-groups."""
    SL = list(SL)
    EXP = len(SL)
    assert all(8 <= c <= S_CAP and c % 8 == 0 for c in SL)
    Tc = sum(SL)
    offs = [0]
    for c in SL:
        offs.append(offs[-1] + c)
    Smax = max(SL)
    nc = bacc.Bacc("TRN2", target_bir_lowering=False, debug=False, num_devices=NCORES)
    xTe = nc.dram_tensor("xTe", [P, KO1 * Tc], BF16, kind="ExternalInput").ap()
    wrow = nc.dram_tensor("wrow", [1, Tc], F32, kind="ExternalInput").ap()
    w1s = nc.dram_tensor(
        "w1s", [EXP, NG1, P, WG1, KO1, P], BF16, kind="ExternalInput"
    ).ap()
    b1s = nc.dram_tensor("b1s", [P, EXP, FO1], F32, kind="ExternalInput").ap()
    w2s = nc.dram_tensor(
        "w2s", [EXP, NG2, P, WG2, KO2, P], BF16, kind="ExternalInput"
    ).ap()
    b2s = nc.dram_tensor("b2s", [P, EXP, FO2], F32, kind="ExternalInput").ap()
    oT = nc.dram_tensor("oT", [P, FO2, Tc], F32, kind="ExternalOutput").ap()

    import contextlib

    with tile.TileContext(nc) as tc:
        with contextlib.ExitStack() as ctx:
            pers = ctx.enter_context(tc.tile_pool(name="pers", bufs=1))
            persb = ctx.enter_context(tc.tile_pool(name="persb", bufs=2))
            psA = ctx.enter_context(tc.tile_pool(name="psA", bufs=3, space="PSUM"))
            w1pool = ctx.enter_context(tc.tile_pool(name="w1p", bufs=2))
            w2pool = ctx.enter_context(tc.tile_pool(name="w2p", bufs=2))
            hpool = ctx.enter_context(tc.tile_pool(name="hp", bufs=1))
            spool = ctx.enter_context(tc.tile_pool(name="sp", bufs=2))
            opool = ctx.enter_context(tc.tile_pool(name="op", bufs=2))
            psB = ctx.enter_context(tc.tile_pool(name="psB", bufs=3, space="PSUM"))
            for _rep in range(reps):
                xTe_s = pers.tile([P, KO1 * Tc], BF16, tag="xTe")
                b1s_s = persb.tile([P, EXP, FO1], F32, tag="b1s")
                b2s_s = persb.tile([P, EXP, FO2], F32, tag="b2s")
                wb2 = pers.tile([P, Tc], F32, tag="wb2")

                # small head DMAs first so the PE isn't stalled behind the
                # bulk xTe transfer (same queue is FIFO)
                ones_sb = pers.tile([1, P], F32, tag="ones")
                nc.vector.memset(ones_sb[:], 1.0)
                wrow_s = persb.tile([1, Tc], F32, tag="wrow")
                nc.scalar.dma_start(wrow_s[:], wrow)
                nc.scalar.dma_start(b1s_s[:], b1s)
                nc.scalar.dma_start(b2s_s[:], b2s)
                # slot-major transfer order: slot j's tokens land long before
                # the PE reaches slot j
                for j in range(EXP):
                    nc.scalar.dma_start(
                        xTe_s[:, KO1 * offs[j] : KO1 * offs[j + 1]],
                        xTe[:, KO1 * offs[j] : KO1 * offs[j + 1]],
                    )
                for a, b in _chunks(Tc):
                    pw = psA.tile([P, 512], F32, tag="psA")
                    nc.tensor.matmul(
                        pw[:, : b - a],
                        lhsT=ones_sb[:],
                        rhs=wrow_s[:, a:b],
                        start=True,
                        stop=True,
                    )
                    nc.scalar.mul(wb2[:, a:b], pw[:, : b - a], 0.5)  # wb2 = w/2

                for j in range(EXP):
                    S = SL[j]
                    t0 = offs[j]
                    cks = _even_chunks(S)
                    hbig = hpool.tile([P, KO2 * Smax], BF16, tag="ht")
                    for g in range(NG1):
                        w1t = w1pool.tile([P, WG1, KO1, P], BF16, tag="w1t")
                        nc.sync.dma_start(w1t[:], w1s[j, g])
                        for q in range(WG1):
                            fo = WG1 * g + q
                            for a, b in cks:
                                pa = psA.tile([P, 512], F32, tag="psA")
                                for ko in range(KO1):
                                    base = KO1 * t0 + ko * S
                                    nc.tensor.matmul(
                                        pa[:, : b - a],
                                        lhsT=w1t[:, q, ko, :],
                                        rhs=xTe_s[:, base + a : base + b],
                                        start=(ko == 0),
                                        stop=(ko == KO1 - 1),
                                    )
                                nc.scalar.activation(
                                    hbig[:, fo * S + a : fo * S + b],
                                    pa[:, : b - a],
                                    GELU,
                                    bias=b1s_s[:, j, fo : fo + 1],
                                )
                    for g2 in range(NG2):
                        w2t = w2pool.tile([P, WG2, KO2, P], BF16, tag="w2t")
                        nc.sync.dma_start(w2t[:], w2s[j, g2])
                        for q in range(WG2):
                            fo2 = WG2 * g2 + q
                            obuf = opool.tile([P, Smax], F32, tag="obuf")
                            for a, b in cks:
                                pb = psB.tile([P, 512], F32, tag="psB")
                                for ko in range(KO2):
                                    nc.tensor.matmul(
                                        pb[:, : b - a],
                                        lhsT=w2t[:, q, ko, :],
                                        rhs=hbig[:, ko * S + a : ko * S + b],
                                        start=(ko == 0),
                                        stop=(ko == KO2 - 1),
                                    )
                                st = spool.tile([P, 512], F32, tag="st")
                                # tanh(0.5*o + 0.5*b2)  (b2s input is pre-halved)
                                nc.scalar.activation(
                                    st[:, : b - a],
                                    pb[:, : b - a],
                                    AF.Tanh,
                                    bias=b2s_s[:, j, fo2 : fo2 + 1],
                                    scale=0.5,
                                )
                                # out = w*sigmoid(o) = wb2 + wb2*tanh
                                nc.vector.tensor_tensor(
                                    obuf[:, a:b],
                                    st[:, : b - a],
                                    wb2[:, t0 + a : t0 + b],
                                    OP.mult,
                                )
                                nc.vector.tensor_tensor(
                                    obuf[:, a:b],
                                    obuf[:, a:b],
                                    wb2[:, t0 + a : t0 + b],
                                    OP.add,
                                )
                            nc.gpsimd.dma_start(
                                oT[:, fo2, t0 : t0 + S], obuf[:, :S]
                            )
    nc.compile()
    return nc


def _host_exact_logits(inputs, rows):
    """Exact fp32 gate logits for the given token rows."""
    c = np.asarray(inputs["combined"], np.float32)[rows]
    gh = _gelu32(c @ np.asarray(inputs["gate_w1"], np.float32)
                 + np.asarray(inputs["gate_b1"], np.float32))
    return (gh @ np.asarray(inputs["gate_w2"], np.float32)
            + np.asarray(inputs["gate_b2"], np.float32))


# ---------- slot-profile selection: minimize PE cost under bin packing ----
#
# Cost model (per core, ns): each token costs 432 PE cycles = 180 ns, and
# each (weight-pass x token-chunk) costs a flat ~48 ns of serialized
# LDWEIGHTS in the self-loading matmul => 432*48 ~ 20.7 us per slot-chunk.
# So J(SL) = 180*sum(SL) + 20700*sum(ceil(SL_j/512)), minimized subject to
# the experts packing into 8 copies of SL with <=1 expert per bin.

_PROFILE_SEEDS = [(480, 456, 432, 360, 352)]  # trial profile B


def _bundles(c, SL):
    """Pareto-minimal bin bundles (per-size counts) covering c tokens."""
    import itertools

    E = len(SL)
    rng = [range(0, min(NCORES, -(-c // SL[j])) + 1) for j in range(E)]
    combos = []
    for n in itertools.product(*rng):
        if sum(n) <= NCORES and sum(nj * s for nj, s in zip(n, SL)) >= c:
            combos.append(n)
    combos.sort(key=sum)
    keep = []
    for n in combos:
        if not any(all(m[j] <= n[j] for j in range(E)) and m != n for m in keep):
            keep.append(n)
    return keep


def _pack(cnt, SL):
    """Exact bin-cover DP.  Returns per-expert bundles (counts per size
    class) or None if the experts don't fit 8 copies of SL."""
    E = len(SL)
    if E > 6:
        return None
    order = np.argsort(-np.asarray(cnt))
    dims = (NCORES + 1,) * E
    nstates = int(np.prod(dims))
    reach = np.zeros(nstates, bool)
    reach[np.ravel_multi_index((NCORES,) * E, dims)] = True
    BL, layers = [], [reach]
    for e in order:
        B = _bundles(int(cnt[e]), SL)
        if not B:
            return None
        BL.append(B)
        idx = np.nonzero(reach)[0]
        if not len(idx):
            return None
        coords = np.stack(np.unravel_index(idx, dims), 1)
        new = np.zeros(nstates, bool)
        for b in B:
            c2 = coords - b
            ok = (c2 >= 0).all(1)
            if ok.any():
                new[np.ravel_multi_index(c2[ok].T.tolist(), dims)] = True
        reach = new
        layers.append(reach)
    if not reach.any():
        return None
    fin = np.nonzero(reach)[0][0]
    state = np.array(np.unravel_index(fin, dims))
    rev = []
    for i in range(len(order) - 1, -1, -1):
        found = None
        for b in BL[i]:
            p = state + b
            if (p <= NCORES).all() and layers[i][np.ravel_multi_index(p.tolist(), dims)]:
                found = b
                state = p
                break
        assert found is not None
        rev.append(found)
    rev.reverse()
    asg = [None] * NE
    for i, e in enumerate(order):
        asg[int(e)] = rev[i]
    return asg


def _waterfill_profile(cnt, EXP):
    """Near-equal waterfill pieces -> sorted groups of 8 -> profile + direct
    assignment (always feasible)."""
    k = np.ones(NE, np.int64)
    for _ in range(NCORES * EXP - NE):
        j = np.argmax(-(-cnt // k))
        k[j] += 1
    pieces = []
    for e in range(NE):
        base, rem = divmod(int(cnt[e]), int(k[e]))
        for i in range(int(k[e])):
            pieces.append((base + (1 if i < rem else 0), e))
    pieces.sort(key=lambda t: -t[0])
    SL = tuple(
        max(8, -(-max(pieces[j * NCORES + c][0] for c in range(NCORES)) // 8) * 8)
        for j in range(EXP)
    )
    asg = [[0] * EXP for _ in range(NE)]
    for j in range(EXP):
        for c in range(NCORES):
            asg[pieces[j * NCORES + c][1]][j] += 1
    return SL, asg


def _profile_cost(SL):
    return 180 * sum(SL) + 20700 * sum(-(-s // 512) for s in SL)


def choose_profile(cnt):
    """Pick the cheapest feasible slot profile for the given expert loads."""
    best = None
    for SL in _PROFILE_SEEDS:
        if max(SL) > S_CAP:
            continue
        asg = _pack(cnt, SL)
        if asg is not None:
            j = _profile_cost(SL)
            if best is None or j < best[0]:
                best = (j, tuple(SL), asg)
    for EXP in (5, 4, 6, 8):
        SL, asg = _waterfill_profile(cnt, EXP)
        if max(SL) > S_CAP:
            continue
        j = _profile_cost(SL)
        if best is None or j < best[0]:
            best = (j, SL, asg)
    if best is None:
        return None, None
    return list(best[1]), best[2]


def route(logits):
    """Host softmax/top-2/normalize + slot assignment via profile packing.

    Returns (slots, SL): slots[c][j] = (expert, token_idx[SL_j], weight[SL_j])
    padded with token 0 / weight 0."""
    lg = logits.astype(np.float32)
    m = lg.max(axis=1, keepdims=True)
    p = np.exp(lg - m)
    p /= p.sum(axis=1, keepdims=True)
    order = np.argsort(-p, axis=1, kind="stable")
    i1, i2 = order[:, 0], order[:, 1]
    r = np.arange(lg.shape[0])
    w1 = p[r, i1]
    w2 = p[r, i2]
    s = w1 + w2
    w1, w2 = w1 / s, w2 / s

    toks, wts = [], []
    for e in range(NE):
        t1 = np.nonzero(i1 == e)[0]
        t2 = np.nonzero(i2 == e)[0]
        toks.append(np.concatenate([t1, t2]))
        wts.append(np.concatenate([w1[t1], w2[t2]]).astype(np.float32))
    cnt = np.array([len(t) for t in toks])

    SL, asg = choose_profile(cnt)
    if SL is None:
        return None, None
    EXP = len(SL)
    # instantiate bins: class j has 8 instances (one per core), round-robin
    nxt = [0] * EXP  # next free core per class
    slots = [[None] * EXP for _ in range(NCORES)]
    for e in range(NE):
        pos = 0
        # fill this expert's bins largest-class-first so the partial bin is
        # the smallest one
        for j in range(EXP):
            for _ in range(asg[e][j]):
                c = nxt[j]
                nxt[j] += 1
                take = max(0, min(SL[j], cnt[e] - pos))
                tt = np.zeros(SL[j], np.int64)
                ww = np.zeros(SL[j], np.float32)
                tt[:take] = toks[e][pos : pos + take]
                ww[:take] = wts[e][pos : pos + take]
                slots[c][j] = (e, tt, ww)
                pos += take
        assert pos >= cnt[e], f"expert {e}: {pos} < {cnt[e]}"
    # unused bins -> padding (expert 0, zero weights)
    for j in range(EXP):
        for c in range(nxt[j], NCORES):
            slots[c][j] = (
                0,
                np.zeros(SL[j], np.int64),
                np.zeros(SL[j], np.float32),
            )
    return slots, SL


def build_emaps(inputs, shared, slots, SL):
    bf16 = mybir.dt.np(BF16)
    EXP = len(SL)
    Tc = sum(SL)
    combined = np.asarray(inputs["combined"], np.float32)
    emaps = []
    for c in range(NCORES):
        eids = [slots[c][j][0] for j in range(EXP)]
        ws = np.concatenate([slots[c][j][2] for j in range(EXP)])
        blocks = []
        for j in range(EXP):
            xg = combined[slots[c][j][1]]  # [SL[j], D]
            blocks.append(
                xg.T.reshape(KO1, P, SL[j]).transpose(1, 0, 2).reshape(P, KO1 * SL[j])
            )
        emaps.append(
            {
                "xTe": np.ascontiguousarray(np.concatenate(blocks, axis=1)).astype(
                    bf16
                ),
                "wrow": ws.reshape(1, Tc).astype(np.float32),
                "w1s": np.ascontiguousarray(shared["w1g"][eids]),
                "b1s": np.ascontiguousarray(shared["b1e"][:, eids, :]),
                "w2s": np.ascontiguousarray(shared["w2g"][eids]),
                "b2s": np.ascontiguousarray(shared["b2e"][:, eids, :]),
            }
        )
    return emaps


def kernel_sparse(**inputs):
    shared, xTs, _, xh2s = prep_inputs(inputs)

    if "gate" not in _NC_CACHE:
        _NC_CACHE["gate"] = build_nc_gate()
    ncg = _NC_CACHE["gate"]
    gmaps = gate_inmaps(shared, xh2s)
    gres = run_bass_kernel_spmd(ncg, gmaps, core_ids=list(range(NCORES)))
    logits = np.concatenate(
        [gres.results[c]["lg"].T for c in range(NCORES)]
    )

    # host fixup: recompute ambiguous tokens (top2/3 gap under THETA) exactly
    srt = np.sort(logits, axis=1)
    amb = np.nonzero((srt[:, -2] - srt[:, -3]) < THETA)[0]
    if len(amb):
        logits[amb] = _host_exact_logits(inputs, amb)

    slots, SL = route(logits)
    if SL is None or max(SL) > S_CAP or sum(SL) > 3000:  # pathological routing
        return kernel_dense(**inputs)
    EXP = len(SL)
    Tc = sum(SL)
    offs = [0]
    for ccap in SL:
        offs.append(offs[-1] + ccap)

    key = ("exp", tuple(SL))
    if key not in _NC_CACHE:
        _NC_CACHE[key] = build_nc_exp(SL)
    nce = _NC_CACHE[key]

    emaps = build_emaps(inputs, shared, slots, SL)
    _NC_CACHE["last_emaps"] = emaps
    _NC_CACHE["last_SL"] = SL
    eres = run_bass_kernel_spmd(nce, emaps, core_ids=list(range(NCORES)))

    fused = np.zeros((N, E), np.float32)
    for c in range(NCORES):
        rows = eres.results[c]["oT"].transpose(2, 1, 0).reshape(Tc, E)
        for j in range(EXP):
            # np.add.at: padding reuses token 0 with an all-zero weight row
            np.add.at(fused, slots[c][j][1], rows[offs[j] : offs[j + 1]])
    return fused


MODE = "sparse"


def kernel(**inputs):
    if MODE == "sparse":
        try:
            return kernel_sparse(**inputs)
        except Exception:
            return kernel_dense(**inputs)
    return kernel_dense(**inputs)


if __name__ == "__main__":  # dev smoke test only; harness imports kernel()
    import reference  # noqa: PLC0415 -- not needed when imported as a module

    inputs = {k: np.asarray(v) for k, v in reference.setup_inputs().items()}
    out = kernel(**inputs)
    print(out.shape, out.dtype)
